# revision 9
# baseline (speedup 1.0000x reference)
"""Bass/Trainium2 kernel for nn_BlockGNN (2-layer GATv2 + MLP) on 8 NeuronCores.

Strategy (per spec sharding hint):
  - Nodes partitioned across 8 cores by destination (6250 nodes/core).
  - Edges routed to the core owning their dst; self-loops appended; packed
    into 98 windows of 64 dst-nodes, each window padded to a uniform KT
    tiles of 128 edge slots (SPMD: same program on all cores).
  - Host performs the halo gather: for every edge slot it gathers the
    source node's raw features into a feature-major slab (x[src].T),
    uploaded per layer. A combined fp8 slab zrhs = [selT(64) | eaT(32) |
    ones(1)] lets one matmul add xr[dst] + ea@We + (bl+br) to z.
  - Per layer (one SPMD launch per layer; host re-shards between layers),
    per macro of up to 4 edge tiles (512 slots), on device:
      PE builds z feature-major [feat, slot] in PSUM with two wide
        matmuls (moving dim 512): Wl.T@xgT + [xr|We|brv].T@zrhs,
      ACT computes am = |att| * leaky_relu(z) in one Prelu op (the
        per-partition scale |att| commutes with leaky_relu),
      PE reduces am over features per head via small sign-mask matmuls
        producing edge-major logits [slot, 4],
      ACT exponentiates into comb's denominator columns, DVE multiplies
        the edge-major value projection vq (PE, Wl) by ex into comb,
      and PE accumulates [sum ex*xl | sum ex] per dst into PSUM via the
        one-hot sel matmuls.
    PE work is software-pipelined two macros deep (z/vq of macro g,
    alpha of g-1, segment-sum of g-2) across window boundaries so the
    tensor engine never stalls.
  - Node stage per window: divide by denominators, +bias, PE transpose
    into a 4-window batch buffer; every 4 windows one float32r 2-matmul
    MLP (moving dim 256 runs at full PE speed with fp32-class accuracy).
"""

import os
import sys
import time

import numpy as np

os.environ.setdefault("MYCRO_LOCAL_CACHE", "1")

for _p in ("/opt/trn_rl_repo",):
    if os.path.isdir(_p) and _p not in sys.path:
        sys.path.append(_p)

import concourse.bass as bass
import concourse.bacc as bacc
import concourse.mybir as mybir
import concourse.tile as tile
from concourse.bass import AP
from concourse.bass_utils import run_bass_kernel_spmd

F32 = mybir.dt.float32
F32R = mybir.dt.float32r
BF16 = mybir.dt.bfloat16
FP8 = mybir.dt.float8e4

NPF32 = np.float32
NPBF16 = mybir.dt.np(BF16)
NPFP8 = mybir.dt.np(FP8)

# Problem constants
N, E, D, H, CDIM, L = 50000, 800000, 128, 4, 32, 2
P = 128
WDST = 64          # dst nodes per window
NCORES = 8
NEG = 0.2
MLPB = 4           # windows per MLP batch

EDGE_DT = BF16
NP_EDGE = NPBF16
SEL_DT = FP8
NP_SEL = NPFP8
TRACE = bool(int(os.environ.get("KTRACE", "0")))

LAST_EXEC_NS = []     # per-launch exec_time_ns when tracing
LAST_RESULTS = []     # per-launch BassKernelResults when tracing


def _install_ntff_hook():
    """Provide antenv.axon_hooks (NTFF profiling via the axon PJRT .so)
    when the image lacks it, so run_bass_kernel_spmd(trace=True) works."""
    try:
        import antenv.axon_hooks  # noqa: F401
        return
    except ImportError:
        pass
    import contextlib
    import ctypes
    import types

    try:
        import antenv
    except ImportError:
        return
    so_path = "/opt/axon/libaxon_pjrt.so"
    if not os.path.exists(so_path):
        return
    lib = ctypes.CDLL(so_path)
    if not hasattr(lib, "axon_start_nrt_profile"):
        return
    lib.axon_start_nrt_profile.argtypes = [
        ctypes.POINTER(ctypes.c_int64),
        ctypes.c_size_t,
    ]
    lib.axon_start_nrt_profile.restype = ctypes.c_int64
    lib.axon_stop_nrt_profile.argtypes = [ctypes.c_char_p]
    lib.axon_stop_nrt_profile.restype = ctypes.c_int64

    @contextlib.contextmanager
    def _hook(output_dir, device_ids):
        import jax

        jax.devices()
        if device_ids:
            ids = (ctypes.c_int64 * len(device_ids))(*device_ids)
            rc = lib.axon_start_nrt_profile(ids, len(device_ids))
        else:
            rc = lib.axon_start_nrt_profile(None, 0)
        if rc != 0:
            raise RuntimeError(f"axon_start_nrt_profile rc={rc}")
        try:
            yield
        finally:
            n = lib.axon_stop_nrt_profile(str(output_dir).encode())
            print(f"ntff profile: {n} file(s) -> {output_dir}", file=sys.stderr)

    mod = types.ModuleType("antenv.axon_hooks")
    _state = {"hook": _hook}
    mod.get_axon_ntff_profile_hook = lambda: _state["hook"]
    mod.set_axon_ntff_profile_hook = lambda h: _state.update(hook=h)
    sys.modules["antenv.axon_hooks"] = mod
    antenv.axon_hooks = mod


if TRACE:
    _install_ntff_hook()


def _bcast_last(ap: AP, n: int) -> AP:
    """Append a stride-0 trailing dim of size n to an AP."""
    return AP(ap.tensor, ap.offset, [list(p) for p in ap.ap] + [[0, n]])


def build_layer_nc(cfg, enable_asserts=False):
    """Build the single-layer SPMD program. cfg: nwin, kt (tiles/window)."""
    NWIN, KT = cfg["nwin"], cfg["kt"]
    NLOCP = NWIN * WDST
    MACROS = []
    j0 = 0
    while j0 < KT:
        wdt = min(4, KT - j0)
        MACROS.append((j0, wdt))
        j0 += wdt
    NMAC = len(MACROS)
    assert NMAC >= 3

    nc = bacc.Bacc(
        "TRN2",
        target_bir_lowering=False,
        debug=False,
        enable_asserts=enable_asserts,
        num_devices=cfg.get("ncores", NCORES),
    )

    # ---- inputs ----
    xgT = nc.dram_tensor("xgT", [P, NWIN * KT * P], EDGE_DT,
                         kind="ExternalInput").ap()
    xTloc = nc.dram_tensor("xTloc", [P, NLOCP], BF16, kind="ExternalInput").ap()
    Wl_b = nc.dram_tensor("Wl_b", [P, P], EDGE_DT, kind="ExternalInput").ap()
    Wr = nc.dram_tensor("Wr", [P, P], BF16, kind="ExternalInput").ap()
    w1 = nc.dram_tensor("w1", [P, P], F32R, kind="ExternalInput").ap()
    w2 = nc.dram_tensor("w2", [P, P], F32R, kind="ExternalInput").ap()
    webrv = nc.dram_tensor("webrv", [CDIM + 1, P], BF16,
                           kind="ExternalInput").ap()
    attcol = nc.dram_tensor("attcol", [P, 1], F32, kind="ExternalInput").ap()
    sgn4 = nc.dram_tensor("sgn4", [P, 4], BF16, kind="ExternalInput").ap()
    i64f = nc.dram_tensor("i64f", [WDST, WDST], F32, kind="ExternalInput").ap()
    b1c = nc.dram_tensor("b1c", [P, 1], F32, kind="ExternalInput").ap()
    b2c = nc.dram_tensor("b2c", [P, 1], F32, kind="ExternalInput").ap()
    bgc = nc.dram_tensor("bgc", [P, 1], F32, kind="ExternalInput").ap()
    seld = nc.dram_tensor("seld", [P, NWIN * KT * WDST], SEL_DT,
                          kind="ExternalInput").ap()
    zrhsd = nc.dram_tensor("zrhsd", [WDST + CDIM + 1, NWIN * KT * P], BF16,
                           kind="ExternalInput").ap()
    xoutT = nc.dram_tensor("xoutT", [P, NLOCP], F32, kind="ExternalOutput").ap()

    AF = mybir.ActivationFunctionType
    OP = mybir.AluOpType
    KMIX = WDST + CDIM + 1  # 97: xr rows + We rows + brv row

    with tile.TileContext(nc) as tc:
        with (
            tc.tile_pool(name="const", bufs=1) as cpool,
            tc.tile_pool(name="xwp", bufs=1) as xwp,
            tc.tile_pool(name="win", bufs=3) as wpool,
            tc.tile_pool(name="edge", bufs=3) as epool,
            tc.tile_pool(name="psZ", bufs=2, space="PSUM") as psZ,
            tc.tile_pool(name="psV", bufs=2, space="PSUM") as psV,
            tc.tile_pool(name="psM", bufs=1, space="PSUM") as psM,
            tc.tile_pool(name="psO", bufs=1, space="PSUM") as psO,
            tc.tile_pool(name="psE", bufs=1, space="PSUM") as psE,
        ):
            # ---- load constants/weights to SBUF ----
            def cload(ap, shape, dt, tag):
                t = cpool.tile(shape, dt, tag=tag)
                nc.sync.dma_start(out=t[:], in_=ap)
                return t

            Wlb_s = cload(Wl_b, [P, P], EDGE_DT, tag="Wlb_s")
            Wr_s = cload(Wr, [P, P], BF16, tag="Wr_s")
            w1_s = cload(w1, [P, P], F32R, tag="w1_s")
            w2_s = cload(w2, [P, P], F32R, tag="w2_s")
            attc_s = cload(attcol, [P, 1], F32, tag="attc_s")
            sgn_s = cload(sgn4, [P, 4], BF16, tag="sgn_s")
            i64f_s = cload(i64f, [WDST, WDST], F32, tag="i64f_s")
            b1c_s = cload(b1c, [P, 1], F32, tag="b1c_s")
            b2c_s = cload(b2c, [P, 1], F32, tag="b2c_s")
            bgc_s = cload(bgc, [P, 1], F32, tag="bgc_s")

            # two manually-alternated stationary tiles [97,128]: rows 0:64
            # get xr per window, rows 64:97 hold We|brv (DMA'd once)
            xw_tiles = []
            for t in range(2):
                xw = xwp.tile([KMIX, P], EDGE_DT, tag=f"xw{t}")
                nc.sync.dma_start(out=xw[WDST:KMIX, :], in_=webrv)
                xw_tiles.append(xw)

            # ---- global macro list (cross-window software pipeline) ----
            GMACS = []
            for w in range(NWIN):
                for mi, (j0, MW) in enumerate(MACROS):
                    GMACS.append((w, mi, j0, MW))
            G = len(GMACS)

            gstate = {}
            wtiles = {}
            pair_tiles = {}
            pending_epi = []
            mlp_ctx = {"gTall": None, "w0": None, "nw": 0}

            def stage_alpha(g):
                """Emit minis(g) + exp(g) + comb-mult(g)."""
                st = gstate[g]
                MW, vq, am, comb = st["MW"], st["vq"], st["am"], st["comb"]
                mini = psM.tile([P, 16], F32, tag="mini")
                for u in range(MW):
                    nc.tensor.matmul(
                        out=mini[:, u * 4 : (u + 1) * 4],
                        lhsT=am[:, u * P : (u + 1) * P],
                        rhs=sgn_s[:],
                        start=(u == 0),
                        stop=(u == MW - 1),
                    )
                comb_v = comb[:, : MW * 132].rearrange("p (b f) -> p b f", f=132)
                nc.scalar.activation(
                    comb_v[:, :, P : P + 4],
                    mini[:, : MW * 4].rearrange("p (b h) -> p b h", h=4),
                    AF.Exp,
                )
                nc.vector.tensor_tensor(
                    comb_v[:, :, 0:P].rearrange("p b (h c) -> p b h c", c=CDIM),
                    vq[:, : MW * P].rearrange("p (b h c) -> p b h c",
                                              b=MW, c=CDIM),
                    _bcast_last(comb_v[:, :, P : P + 4], CDIM),
                    op=OP.mult,
                )

            def stage_out12(g):
                """Emit segment-sum accumulation for macro g."""
                st = gstate[g]
                mi, j0, MW = st["mi"], st["j0"], st["MW"]
                sel_sb, out12, comb = st["sel_sb"], st["out12"], st["comb"]
                so = st["soff"]
                for u in range(MW):
                    j = j0 + u
                    nc.tensor.matmul(
                        out=out12[:],
                        lhsT=sel_sb[:, so + j * WDST : so + (j + 1) * WDST],
                        rhs=comb[:, u * 132 : (u + 1) * 132],
                        start=(mi == 0 and u == 0),
                        stop=(mi == NMAC - 1 and u == MW - 1),
                    )
                if mi == NMAC - 1:
                    pending_epi.append(st["w"])

            def emit_mlp_batch():
                """float32r MLP over the accumulated transpose batch."""
                gTall, w0, nw = mlp_ctx["gTall"], mlp_ctx["w0"], mlp_ctx["nw"]
                if gTall is None:
                    return
                mlp_ctx["gTall"] = None
                S = nw * WDST
                gTb = wpool.tile([P, MLPB * WDST], F32R, tag="gTb")
                nc.scalar.activation(gTb[:, :S], gTall[:, :S], AF.Identity,
                                     bias=bgc_s[:])
                y1_ps = psE.tile([P, MLPB * WDST], F32, tag="epi")
                nc.tensor.matmul(out=y1_ps[:, :S],
                                 lhsT=w1_s[:],
                                 rhs=gTb[:, :S],
                                 start=True, stop=True)
                y1s = wpool.tile([P, MLPB * WDST], F32R, tag="y1s")
                nc.scalar.activation(y1s[:, :S], y1_ps[:, :S], AF.Relu,
                                     bias=b1c_s[:])
                y2_ps = psE.tile([P, MLPB * WDST], F32, tag="epi")
                nc.tensor.matmul(out=y2_ps[:, :S],
                                 lhsT=w2_s[:],
                                 rhs=y1s[:, :S],
                                 start=True, stop=True)
                xo = wpool.tile([P, MLPB * WDST], F32, tag="xo")
                nc.scalar.activation(xo[:, :S], y2_ps[:, :S], AF.Identity,
                                     bias=b2c_s[:])
                nc.sync.dma_start(
                    out=xoutT[:, w0 * WDST : w0 * WDST + S], in_=xo[:, :S]
                )

            def emit_epilogue(w):
                out12 = wtiles[w]["out12"]
                de = wpool.tile([WDST, 4], F32, tag="de")
                nc.vector.tensor_scalar(de[:], out12[:, P : P + 4],
                                        1e-16, None, OP.add)
                rc = wpool.tile([WDST, 4], F32, tag="rc")
                nc.vector.reciprocal(rc[:], de[:])
                gat = wpool.tile([WDST, P], F32, tag="gat")
                for h in range(H):
                    nc.vector.tensor_scalar(
                        gat[:, h * CDIM : (h + 1) * CDIM],
                        out12[:, h * CDIM : (h + 1) * CDIM],
                        rc[:, h : h + 1],
                        None,
                        OP.mult,
                    )
                if mlp_ctx["gTall"] is None:
                    mlp_ctx["gTall"] = psE.tile([P, MLPB * WDST], F32,
                                                tag="epi", name="gTall")
                    mlp_ctx["w0"] = w
                    mlp_ctx["nw"] = 0
                k = mlp_ctx["nw"]
                nc.tensor.transpose(
                    mlp_ctx["gTall"][:, k * WDST : (k + 1) * WDST],
                    gat[:], i64f_s[:],
                )
                mlp_ctx["nw"] = k + 1
                del wtiles[w]
                if mlp_ctx["nw"] == MLPB:
                    emit_mlp_batch()

            for g in range(G + 2):
                if g < G:
                    w, mi, j0, MW = GMACS[g]
                    if mi == 0:
                        # ---- window top: pair-batched DMAs + xr ----
                        if w % 2 == 0:
                            nw2 = min(2, NWIN - w)
                            xgT_sb = wpool.tile([P, 2 * KT * P], EDGE_DT,
                                                tag="xgT", name="xgT_sb")
                            nc.sync.dma_start(
                                out=xgT_sb[:, : nw2 * KT * P],
                                in_=xgT[:, w * KT * P : (w + nw2) * KT * P],
                            )
                            zrhs_sb = wpool.tile([KMIX, 2 * KT * P], BF16,
                                                 tag="zrhs", name="zrhs_sb")
                            nc.sync.dma_start(
                                out=zrhs_sb[:, : nw2 * KT * P],
                                in_=zrhsd[:, w * KT * P : (w + nw2) * KT * P],
                            )
                            sel_sb = wpool.tile([P, 2 * KT * WDST], SEL_DT,
                                                tag="sel", name="sel_sb")
                            nc.sync.dma_start(
                                out=sel_sb[:, : nw2 * KT * WDST],
                                in_=seld[:, w * KT * WDST
                                         : (w + nw2) * KT * WDST],
                            )
                            pair_tiles["xgT"] = xgT_sb
                            pair_tiles["zrhs"] = zrhs_sb
                            pair_tiles["sel"] = sel_sb
                        if w % 4 == 0:
                            nw4 = min(4, NWIN - w)
                            xtl_sb = wpool.tile([P, 4 * WDST], BF16,
                                                tag="xtl", name="xtl_sb")
                            nc.sync.dma_start(
                                out=xtl_sb[:, : nw4 * WDST],
                                in_=xTloc[:, w * WDST : (w + nw4) * WDST],
                            )
                            pair_tiles["xtl"] = xtl_sb
                        xr_ps = psM.tile([WDST, P], F32, tag="xr")
                        nc.tensor.matmul(
                            out=xr_ps[:],
                            lhsT=pair_tiles["xtl"][
                                :, (w % 4) * WDST : (w % 4 + 1) * WDST],
                            rhs=Wr_s[:], start=True, stop=True)
                        xw = xw_tiles[w % 2]
                        nc.vector.tensor_copy(xw[0:WDST, :], xr_ps[:])
                        out12 = psO.tile([WDST, 132], F32, tag="o12")
                        wtiles[w] = dict(xgT_sb=pair_tiles["xgT"],
                                         zrhs_sb=pair_tiles["zrhs"],
                                         sel_sb=pair_tiles["sel"],
                                         xw=xw, out12=out12,
                                         zoff=(w % 2) * KT * P,
                                         soff=(w % 2) * KT * WDST)

                    wt = wtiles[w]
                    zo, so = wt["zoff"], wt["soff"]
                    S = MW * P
                    # z feature-major: Wl.T@xgT + [xr|We|brv].T@zrhs
                    zq = psZ.tile([P, 4 * P], F32, tag="zq")
                    nc.tensor.matmul(
                        out=zq[:, :S],
                        lhsT=Wlb_s[:],
                        rhs=wt["xgT_sb"][:, zo + j0 * P : zo + j0 * P + S],
                        start=True, stop=False,
                    )
                    nc.tensor.matmul(
                        out=zq[:, :S],
                        lhsT=wt["xw"][:],
                        rhs=wt["zrhs_sb"][:, zo + j0 * P : zo + j0 * P + S],
                        start=False, stop=True,
                    )
                    # am = |att| * leaky_relu(z)  (per-partition scale)
                    am = epool.tile([P, 4 * P], EDGE_DT, tag="am")
                    nc.scalar.activation(am[:, :S], zq[:, :S], AF.Prelu,
                                         scale=attc_s[:], alpha=NEG)
                    # value projection vq = xg@Wl, edge-major
                    vq = psV.tile([P, 4 * P], F32, tag="vq")
                    for u in range(MW):
                        j = j0 + u
                        nc.tensor.matmul(
                            out=vq[:, u * P : (u + 1) * P],
                            lhsT=wt["xgT_sb"][:, zo + j * P
                                              : zo + (j + 1) * P],
                            rhs=Wlb_s[:],
                            start=(u == 0),
                            stop=(u == MW - 1),
                        )
                    comb = epool.tile([P, 4 * 132], EDGE_DT, tag="comb")
                    gstate[g] = dict(w=w, mi=mi, j0=j0, MW=MW, vq=vq, am=am,
                                     comb=comb, sel_sb=wt["sel_sb"],
                                     out12=wt["out12"], soff=so)

                # epilogues before the next out12-start (shared PSUM bank)
                while pending_epi:
                    emit_epilogue(pending_epi.pop(0))
                if g >= 1 and g - 1 < G:
                    stage_alpha(g - 1)
                if g >= 2:
                    stage_out12(g - 2)
                    del gstate[g - 2]
            while pending_epi:
                emit_epilogue(pending_epi.pop(0))
            emit_mlp_batch()

    nc.compile()
    return nc


# ----------------------------------------------------------------------------
# Host-side preprocessing
# ----------------------------------------------------------------------------

def _preprocess(edge_index, edge_attr, ncores, nloc, nwin):
    """Route edges per core into gather-ready slot arrays.

    Node->window assignment is degree-balanced (LPT) so every window has a
    near-equal edge count, minimizing the uniform KT tile budget. Slot s of
    window w: tile j = s // 128, edge lane q = s % 128. Returns per-core
    dicts with src_slot (for the per-layer host halo gather), one-hot
    sel, the combined fp8 z-rhs slab [selT | eaT | ones], and the node
    permutation, plus kt.
    """
    src = np.ascontiguousarray(edge_index[0]).astype(np.int64)
    dst = np.ascontiguousarray(edge_index[1]).astype(np.int64)
    n = nloc * ncores
    ea = np.ascontiguousarray(edge_attr, dtype=np.float32)

    deg = np.bincount(dst, minlength=n).astype(np.float32)
    order = np.argsort(dst, kind="stable")
    dst_s = dst[order]
    src_s = src[order]
    ea_s = ea[order]
    cs = np.concatenate(
        [np.zeros((1, ea.shape[1]), np.float64), np.cumsum(ea_s, 0, dtype=np.float64)]
    )
    starts = np.searchsorted(dst_s, np.arange(n))
    ends = np.searchsorted(dst_s, np.arange(n) + 1)
    loop_attr = ((cs[ends] - cs[starts]) / np.maximum(deg, 1.0)[:, None]).astype(
        np.float32
    )

    import heapq

    cores = []
    maxcnt = 0
    for c in range(ncores):
        base = c * nloc
        # edges of this core (dst-local), self-loops appended
        lo, hi = starts[base], ends[base + nloc - 1]
        s2 = np.concatenate([src_s[lo:hi], np.arange(base, base + nloc)])
        dl = np.concatenate([dst_s[lo:hi], np.arange(base, base + nloc)]) - base
        e2 = np.concatenate([ea_s[lo:hi], loop_attr[base : base + nloc]], 0)

        # LPT balance: assign local nodes (weight = deg+1) to nwin windows
        w_of = np.empty(nloc, np.int64)
        pos_of = np.empty(nloc, np.int64)
        wdeg = (deg[base : base + nloc] + 1.0).astype(np.int64)
        heap = [(0, w, 0) for w in range(nwin)]  # (total, window, fill)
        heapq.heapify(heap)
        for node in np.argsort(-wdeg):
            tot, w, fill = heapq.heappop(heap)
            w_of[node] = w
            pos_of[node] = fill
            fill += 1
            tot += int(wdeg[node])
            if fill < WDST:
                heapq.heappush(heap, (tot, w, fill))
            else:
                heapq.heappush(heap, (1 << 60, w, fill))
        we = w_of[dl]
        pe_ = pos_of[dl]
        o = np.argsort(we, kind="stable")
        s2, e2, we, pe_ = s2[o], e2[o], we[o], pe_[o]
        wstart = np.searchsorted(we, np.arange(nwin))
        wend = np.searchsorted(we, np.arange(nwin) + 1)
        cnts = wend - wstart
        maxcnt = max(maxcnt, int(cnts.max()))
        # node permutation: slot w*WDST+pos holds local node id
        nl_flat = np.zeros(nwin * WDST, np.int64)
        nl_flat[w_of * WDST + pos_of] = np.arange(nloc)
        used = np.zeros(nwin * WDST, bool)
        used[w_of * WDST + pos_of] = True
        cores.append((s2, e2, pe_, wstart, cnts, nl_flat, used))

    kt = -(-maxcnt // P)
    S = kt * P

    data = []
    for (s2, e2, pe_, wstart, cnts, nl_flat, used) in cores:
        nslot = nwin * S
        src_slot = np.zeros(nslot, np.int64)
        dstw_slot = np.full(nslot, -1, np.int64)
        ea_slot = np.zeros((nslot, CDIM), np.float32)
        idx = np.concatenate([np.arange(cnts[w]) + w * S for w in range(nwin)])
        src_slot[idx] = s2
        dstw_slot[idx] = pe_
        ea_slot[idx] = e2

        dw = dstw_slot.reshape(nwin, kt, P)  # [w, j, q]
        # sel global 2D [q, w*kt*WDST + j*WDST + p]
        sel = (dw[:, :, :, None] == np.arange(WDST)[None, None, None, :])
        sel = np.ascontiguousarray(
            sel.transpose(2, 0, 1, 3).reshape(P, nwin * kt * WDST)
        ).astype(NP_SEL)
        # zrhs global 2D [0:64 selT | 64:96 eaT | 96 ones, w*kt*128 + j*128+q]
        zrhs = np.zeros((WDST + CDIM + 1, nwin * kt * P), np.float32)
        selT = (dw[:, :, None, :] == np.arange(WDST)[None, None, :, None])
        zrhs[0:WDST, :] = selT.transpose(2, 0, 1, 3).reshape(WDST, nwin * kt * P)
        zrhs[WDST : WDST + CDIM, :] = ea_slot.T
        zrhs[WDST + CDIM, :] = 1.0
        data.append(dict(src_slot=src_slot, seld=sel,
                         zrhsd=zrhs.astype(NPBF16),
                         nl_flat=nl_flat, used=used))
    return data, kt


def _layer_weight_maps(inputs, layer, att):
    """Shared (same for all cores) weight/const arrays for one layer."""
    i = layer
    attf = att[i].reshape(-1).astype(np.float32)  # [128], f = h*32+c
    sgn = np.zeros((P, H), np.float32)
    for h in range(H):
        sgn[h * CDIM : (h + 1) * CDIM, h] = np.sign(
            attf[h * CDIM : (h + 1) * CDIM]
        )
    webrv = np.concatenate(
        [
            np.asarray(inputs["We"][i], np.float32),           # [32,128]
            (np.asarray(inputs["br"][i]) + np.asarray(inputs["bl"][i]))
            .reshape(1, P)
            .astype(np.float32),
        ],
        axis=0,
    )
    m = dict(
        Wl_b=np.ascontiguousarray(inputs["Wl"][i]).astype(NP_EDGE),
        Wr=np.ascontiguousarray(inputs["Wr"][i]).astype(NPBF16),
        w1=np.ascontiguousarray(inputs["w1"][i]).astype(NPF32),
        w2=np.ascontiguousarray(inputs["w2"][i]).astype(NPF32),
        webrv=webrv.astype(NPBF16),
        attcol=np.abs(attf).reshape(P, 1).astype(NPF32),
        sgn4=sgn.astype(NPBF16),
        i64f=np.eye(WDST, dtype=NPF32),
        b1c=np.asarray(inputs["b1"][i]).reshape(P, 1).astype(NPF32),
        b2c=np.asarray(inputs["b2"][i]).reshape(P, 1).astype(NPF32),
        # gat bias + bl (bl rides the normalized softmax weights, sum ~= 1)
        bgc=(np.asarray(inputs["bias"][i]) + np.asarray(inputs["bl"][i]))
        .reshape(P, 1)
        .astype(NPF32),
    )
    return m


_NC_CACHE = {}


def kernel(**inputs):
    nodes = np.asarray(inputs["nodes"], dtype=np.float32)
    edge_index = np.asarray(inputs["edge_index"])
    edge_attr = np.asarray(inputs["edge_attr"], dtype=np.float32)

    n, d = nodes.shape
    assert (n, d) == (N, D)
    nloc = n // NCORES
    nwin = -(-nloc // WDST)

    data, kt = _preprocess(edge_index, edge_attr, NCORES, nloc, nwin)

    key = (nwin, kt, NCORES)
    if key not in _NC_CACHE:
        _NC_CACHE[key] = build_layer_nc(dict(nwin=nwin, kt=kt, ncores=NCORES))
    nc = _NC_CACHE[key]

    x_curr = np.ascontiguousarray(nodes.T)  # [128, n] f32

    for layer in range(L):
        wmap = _layer_weight_maps(inputs, layer, np.asarray(inputs["att"]))
        xce = x_curr.astype(NP_EDGE)
        in_maps = []
        for c in range(NCORES):
            base = c * nloc
            xTloc = x_curr[:, base + data[c]["nl_flat"]].copy()
            xTloc[:, ~data[c]["used"]] = 0.0
            m = dict(wmap)
            m["xgT"] = np.ascontiguousarray(xce[:, data[c]["src_slot"]])
            m["xTloc"] = xTloc.astype(NPBF16)
            m["seld"] = data[c]["seld"]
            m["zrhsd"] = data[c]["zrhsd"]
            in_maps.append(m)
        res = run_bass_kernel_spmd(
            nc, in_maps, core_ids=list(range(NCORES)), trace=TRACE
        )
        if res.exec_time_ns is not None:
            LAST_EXEC_NS.append(res.exec_time_ns)
        if TRACE:
            LAST_RESULTS.append(res)
        outs = res.results
        x_next = np.zeros((P, n), NPF32)
        for c in range(NCORES):
            xo = outs[c]["xoutT"]
            u = data[c]["used"]
            x_next[:, c * nloc + data[c]["nl_flat"][u]] = xo[:, u]
        x_curr = x_next

    return np.ascontiguousarray(x_curr.T.astype(np.float32))


# revision 10
# speedup vs baseline: 1.8326x; 1.8326x over previous
"""Bass/Trainium2 kernel for nn_BlockGNN (2-layer GATv2 + MLP) on 8 NeuronCores.

Strategy (per spec sharding hint):
  - Nodes partitioned across 8 cores by destination (6250 nodes/core).
  - Edges routed to the core owning their dst; self-loops appended; packed
    into 98 windows of 64 dst-nodes, each window padded to a uniform KT
    tiles of 128 edge slots (SPMD: same program on all cores).
  - Host performs the halo gather: for every edge slot it gathers the
    source node's raw features into a feature-major slab (x[src].T),
    uploaded per layer. A combined fp8 slab zrhs = [selT(64) | eaT(32) |
    ones(1)] lets one matmul add xr[dst] + ea@We + (bl+br) to z.
  - Per layer (one SPMD launch per layer; host re-shards between layers),
    per macro of up to 4 edge tiles (512 slots), on device:
      PE builds z feature-major [feat, slot] in PSUM with two wide
        matmuls (moving dim 512): Wl.T@xgT + [xr|We|brv].T@zrhs,
      ACT computes am = |att| * leaky_relu(z) in one Prelu op (the
        per-partition scale |att| commutes with leaky_relu),
      PE reduces am over features per head via small sign-mask matmuls
        producing edge-major logits [slot, 4],
      ACT exponentiates into comb's denominator columns, DVE multiplies
        the edge-major value projection vq (PE, Wl) by ex into comb,
      and PE accumulates [sum ex*xl | sum ex] per dst into PSUM via the
        one-hot sel matmuls.
    PE work is software-pipelined two macros deep (z/vq of macro g,
    alpha of g-1, segment-sum of g-2) across window boundaries so the
    tensor engine never stalls.
  - Node stage per window: divide by denominators, +bias, PE transpose
    into a 4-window batch buffer; every 4 windows one float32r 2-matmul
    MLP (moving dim 256 runs at full PE speed with fp32-class accuracy).
"""

import os
import sys
import time

import numpy as np

os.environ.setdefault("MYCRO_LOCAL_CACHE", "1")

for _p in ("/opt/trn_rl_repo",):
    if os.path.isdir(_p) and _p not in sys.path:
        sys.path.append(_p)

import concourse.bass as bass
import concourse.bacc as bacc
import concourse.mybir as mybir
import concourse.tile as tile
from concourse.bass import AP
from concourse.bass_utils import run_bass_kernel_spmd

F32 = mybir.dt.float32
F32R = mybir.dt.float32r
BF16 = mybir.dt.bfloat16
FP8 = mybir.dt.float8e4

NPF32 = np.float32
NPBF16 = mybir.dt.np(BF16)
NPFP8 = mybir.dt.np(FP8)

# Problem constants
N, E, D, H, CDIM, L = 50000, 800000, 128, 4, 32, 2
P = 128
WDST = 64          # dst nodes per window
NCORES = 8
NEG = 0.2
MLPB = 4           # windows per MLP batch

EDGE_DT = BF16
NP_EDGE = NPBF16
SEL_DT = FP8
NP_SEL = NPFP8
TRACE = bool(int(os.environ.get("KTRACE", "0")))

LAST_EXEC_NS = []     # per-launch exec_time_ns when tracing
LAST_RESULTS = []     # per-launch BassKernelResults when tracing


def _install_ntff_hook():
    """Provide antenv.axon_hooks (NTFF profiling via the axon PJRT .so)
    when the image lacks it, so run_bass_kernel_spmd(trace=True) works."""
    try:
        import antenv.axon_hooks  # noqa: F401
        return
    except ImportError:
        pass
    import contextlib
    import ctypes
    import types

    try:
        import antenv
    except ImportError:
        return
    so_path = "/opt/axon/libaxon_pjrt.so"
    if not os.path.exists(so_path):
        return
    lib = ctypes.CDLL(so_path)
    if not hasattr(lib, "axon_start_nrt_profile"):
        return
    lib.axon_start_nrt_profile.argtypes = [
        ctypes.POINTER(ctypes.c_int64),
        ctypes.c_size_t,
    ]
    lib.axon_start_nrt_profile.restype = ctypes.c_int64
    lib.axon_stop_nrt_profile.argtypes = [ctypes.c_char_p]
    lib.axon_stop_nrt_profile.restype = ctypes.c_int64

    @contextlib.contextmanager
    def _hook(output_dir, device_ids):
        import jax

        jax.devices()
        if device_ids:
            ids = (ctypes.c_int64 * len(device_ids))(*device_ids)
            rc = lib.axon_start_nrt_profile(ids, len(device_ids))
        else:
            rc = lib.axon_start_nrt_profile(None, 0)
        if rc != 0:
            raise RuntimeError(f"axon_start_nrt_profile rc={rc}")
        try:
            yield
        finally:
            n = lib.axon_stop_nrt_profile(str(output_dir).encode())
            print(f"ntff profile: {n} file(s) -> {output_dir}", file=sys.stderr)

    mod = types.ModuleType("antenv.axon_hooks")
    _state = {"hook": _hook}
    mod.get_axon_ntff_profile_hook = lambda: _state["hook"]
    mod.set_axon_ntff_profile_hook = lambda h: _state.update(hook=h)
    sys.modules["antenv.axon_hooks"] = mod
    antenv.axon_hooks = mod


if TRACE:
    _install_ntff_hook()


def _bcast_last(ap: AP, n: int) -> AP:
    """Append a stride-0 trailing dim of size n to an AP."""
    return AP(ap.tensor, ap.offset, [list(p) for p in ap.ap] + [[0, n]])


def build_layer_nc(cfg, enable_asserts=False):
    """Build the single-layer SPMD program. cfg: nwin, kt (tiles/window)."""
    NWIN, KT = cfg["nwin"], cfg["kt"]
    NLOCP = NWIN * WDST
    MACROS = []
    j0 = 0
    while j0 < KT:
        wdt = min(4, KT - j0)
        MACROS.append((j0, wdt))
        j0 += wdt
    NMAC = len(MACROS)
    assert NMAC >= 3

    nc = bacc.Bacc(
        "TRN2",
        target_bir_lowering=False,
        debug=False,
        enable_asserts=enable_asserts,
        num_devices=cfg.get("ncores", NCORES),
    )

    # ---- inputs ----
    xgT = nc.dram_tensor("xgT", [P, NWIN * KT * P], EDGE_DT,
                         kind="ExternalInput").ap()
    xTloc = nc.dram_tensor("xTloc", [P, NLOCP], BF16, kind="ExternalInput").ap()
    Wl_b = nc.dram_tensor("Wl_b", [P, P], EDGE_DT, kind="ExternalInput").ap()
    Wr = nc.dram_tensor("Wr", [P, P], BF16, kind="ExternalInput").ap()
    w1 = nc.dram_tensor("w1", [P, P], F32R, kind="ExternalInput").ap()
    w2 = nc.dram_tensor("w2", [P, P], F32R, kind="ExternalInput").ap()
    webrv = nc.dram_tensor("webrv", [CDIM + 1, P], BF16,
                           kind="ExternalInput").ap()
    attcol = nc.dram_tensor("attcol", [P, 1], F32, kind="ExternalInput").ap()
    sgn4 = nc.dram_tensor("sgn4", [P, 4], BF16, kind="ExternalInput").ap()
    i64f = nc.dram_tensor("i64f", [WDST, WDST], F32, kind="ExternalInput").ap()
    b1c = nc.dram_tensor("b1c", [P, 1], F32, kind="ExternalInput").ap()
    b2c = nc.dram_tensor("b2c", [P, 1], F32, kind="ExternalInput").ap()
    bgc = nc.dram_tensor("bgc", [P, 1], F32, kind="ExternalInput").ap()
    seld = nc.dram_tensor("seld", [P, NWIN * KT * WDST], SEL_DT,
                          kind="ExternalInput").ap()
    zrhsd = nc.dram_tensor("zrhsd", [P, NWIN * KT * P], BF16,
                           kind="ExternalInput").ap()
    xoutT = nc.dram_tensor("xoutT", [P, NLOCP], F32, kind="ExternalOutput").ap()

    AF = mybir.ActivationFunctionType
    OP = mybir.AluOpType
    KMIX = WDST + CDIM + 1  # 97: xr rows + We rows + brv row

    with tile.TileContext(nc) as tc:
        with (
            tc.tile_pool(name="const", bufs=1) as cpool,
            tc.tile_pool(name="xwp", bufs=1) as xwp,
            tc.tile_pool(name="win", bufs=3) as wpool,
            tc.tile_pool(name="edge", bufs=3) as epool,
            tc.tile_pool(name="psZ", bufs=2, space="PSUM") as psZ,
            tc.tile_pool(name="psV", bufs=2, space="PSUM") as psV,
            tc.tile_pool(name="psM", bufs=1, space="PSUM") as psM,
            tc.tile_pool(name="psO", bufs=1, space="PSUM") as psO,
            tc.tile_pool(name="psE", bufs=1, space="PSUM") as psE,
        ):
            # ---- load constants/weights to SBUF ----
            def cload(ap, shape, dt, tag):
                t = cpool.tile(shape, dt, tag=tag)
                nc.sync.dma_start(out=t[:], in_=ap)
                return t

            Wlb_s = cload(Wl_b, [P, P], EDGE_DT, tag="Wlb_s")
            Wr_s = cload(Wr, [P, P], BF16, tag="Wr_s")
            w1_s = cload(w1, [P, P], F32R, tag="w1_s")
            w2_s = cload(w2, [P, P], F32R, tag="w2_s")
            attc_s = cload(attcol, [P, 1], F32, tag="attc_s")
            sgn_s = cload(sgn4, [P, 4], BF16, tag="sgn_s")
            i64f_s = cload(i64f, [WDST, WDST], F32, tag="i64f_s")
            b1c_s = cload(b1c, [P, 1], F32, tag="b1c_s")
            b2c_s = cload(b2c, [P, 1], F32, tag="b2c_s")
            bgc_s = cload(bgc, [P, 1], F32, tag="bgc_s")

            # two manually-alternated stationary tiles [97,128]: rows 0:64
            # get xr per window, rows 64:97 hold We|brv (DMA'd once)
            xw_tiles = []
            for t in range(2):
                xw = xwp.tile([KMIX, P], EDGE_DT, tag=f"xw{t}")
                nc.sync.dma_start(out=xw[WDST:KMIX, :], in_=webrv)
                xw_tiles.append(xw)

            # ---- global macro list (cross-window software pipeline) ----
            GMACS = []
            for w in range(NWIN):
                for mi, (j0, MW) in enumerate(MACROS):
                    GMACS.append((w, mi, j0, MW))
            G = len(GMACS)

            gstate = {}
            wtiles = {}
            pair_tiles = {}
            pending_epi = []
            mlp_ctx = {"gTall": None, "w0": None, "nw": 0}

            def stage_alpha(g):
                """Emit minis(g) + exp(g) + comb-mult(g)."""
                st = gstate[g]
                MW, vq, am, comb = st["MW"], st["vq"], st["am"], st["comb"]
                mini = psM.tile([P, 16], F32, tag="mini")
                for u in range(MW):
                    nc.tensor.matmul(
                        out=mini[:, u * 4 : (u + 1) * 4],
                        lhsT=am[:, u * P : (u + 1) * P],
                        rhs=sgn_s[:],
                        start=(u == 0),
                        stop=(u == MW - 1),
                    )
                comb_v = comb[:, : MW * 132].rearrange("p (b f) -> p b f", f=132)
                nc.scalar.activation(
                    comb_v[:, :, P : P + 4],
                    mini[:, : MW * 4].rearrange("p (b h) -> p b h", h=4),
                    AF.Exp,
                )
                nc.vector.tensor_tensor(
                    comb_v[:, :, 0:P].rearrange("p b (h c) -> p b h c", c=CDIM),
                    vq[:, : MW * P].rearrange("p (b h c) -> p b h c",
                                              b=MW, c=CDIM),
                    _bcast_last(comb_v[:, :, P : P + 4], CDIM),
                    op=OP.mult,
                )

            def stage_out12(g):
                """Emit segment-sum accumulation for macro g."""
                st = gstate[g]
                mi, j0, MW = st["mi"], st["j0"], st["MW"]
                sel_sb, out12, comb = st["sel_sb"], st["out12"], st["comb"]
                so = st["soff"]
                for u in range(MW):
                    j = j0 + u
                    nc.tensor.matmul(
                        out=out12[:],
                        lhsT=sel_sb[:, so + j * WDST : so + (j + 1) * WDST],
                        rhs=comb[:, u * 132 : (u + 1) * 132],
                        start=(mi == 0 and u == 0),
                        stop=(mi == NMAC - 1 and u == MW - 1),
                    )
                if mi == NMAC - 1:
                    pending_epi.append(st["w"])

            def emit_mlp_batch():
                """float32r MLP over the accumulated transpose batch."""
                gTall, w0, nw = mlp_ctx["gTall"], mlp_ctx["w0"], mlp_ctx["nw"]
                if gTall is None:
                    return
                mlp_ctx["gTall"] = None
                S = nw * WDST
                gTb = wpool.tile([P, MLPB * WDST], F32R, tag="gTb")
                nc.scalar.activation(gTb[:, :S], gTall[:, :S], AF.Identity,
                                     bias=bgc_s[:])
                y1_ps = psE.tile([P, MLPB * WDST], F32, tag="epi")
                nc.tensor.matmul(out=y1_ps[:, :S],
                                 lhsT=w1_s[:],
                                 rhs=gTb[:, :S],
                                 start=True, stop=True)
                y1s = wpool.tile([P, MLPB * WDST], F32R, tag="y1s")
                nc.scalar.activation(y1s[:, :S], y1_ps[:, :S], AF.Relu,
                                     bias=b1c_s[:])
                y2_ps = psE.tile([P, MLPB * WDST], F32, tag="epi")
                nc.tensor.matmul(out=y2_ps[:, :S],
                                 lhsT=w2_s[:],
                                 rhs=y1s[:, :S],
                                 start=True, stop=True)
                xo = wpool.tile([P, MLPB * WDST], F32, tag="xo")
                nc.scalar.activation(xo[:, :S], y2_ps[:, :S], AF.Identity,
                                     bias=b2c_s[:])
                nc.sync.dma_start(
                    out=xoutT[:, w0 * WDST : w0 * WDST + S], in_=xo[:, :S]
                )

            def emit_epilogue(w):
                out12 = wtiles[w]["out12"]
                de = wpool.tile([WDST, 4], F32, tag="de")
                nc.vector.tensor_scalar(de[:], out12[:, P : P + 4],
                                        1e-16, None, OP.add)
                rc = wpool.tile([WDST, 4], F32, tag="rc")
                nc.vector.reciprocal(rc[:], de[:])
                gat = wpool.tile([WDST, P], F32, tag="gat")
                for h in range(H):
                    nc.vector.tensor_scalar(
                        gat[:, h * CDIM : (h + 1) * CDIM],
                        out12[:, h * CDIM : (h + 1) * CDIM],
                        rc[:, h : h + 1],
                        None,
                        OP.mult,
                    )
                if mlp_ctx["gTall"] is None:
                    mlp_ctx["gTall"] = psE.tile([P, MLPB * WDST], F32,
                                                tag="epi", name="gTall")
                    mlp_ctx["w0"] = w
                    mlp_ctx["nw"] = 0
                k = mlp_ctx["nw"]
                nc.tensor.transpose(
                    mlp_ctx["gTall"][:, k * WDST : (k + 1) * WDST],
                    gat[:], i64f_s[:],
                )
                mlp_ctx["nw"] = k + 1
                del wtiles[w]
                if mlp_ctx["nw"] == MLPB:
                    emit_mlp_batch()

            for g in range(G + 2):
                if g < G:
                    w, mi, j0, MW = GMACS[g]
                    if mi == 0:
                        # ---- window top: pair-batched DMAs + xr ----
                        if w % 2 == 0:
                            nw2 = min(2, NWIN - w)
                            xgT_sb = wpool.tile([P, 2 * KT * P], EDGE_DT,
                                                tag="xgT", name="xgT_sb")
                            nc.sync.dma_start(
                                out=xgT_sb[:, : nw2 * KT * P],
                                in_=xgT[:, w * KT * P : (w + nw2) * KT * P],
                            )
                            zrhs_sb = wpool.tile([P, 2 * KT * P], BF16,
                                                 tag="zrhs", name="zrhs_sb")
                            nc.sync.dma_start(
                                out=zrhs_sb[:, : nw2 * KT * P],
                                in_=zrhsd[:, w * KT * P : (w + nw2) * KT * P],
                            )
                            sel_sb = wpool.tile([P, 2 * KT * WDST], SEL_DT,
                                                tag="sel", name="sel_sb")
                            nc.sync.dma_start(
                                out=sel_sb[:, : nw2 * KT * WDST],
                                in_=seld[:, w * KT * WDST
                                         : (w + nw2) * KT * WDST],
                            )
                            pair_tiles["xgT"] = xgT_sb
                            pair_tiles["zrhs"] = zrhs_sb
                            pair_tiles["sel"] = sel_sb
                        if w % 4 == 0:
                            nw4 = min(4, NWIN - w)
                            xtl_sb = wpool.tile([P, 4 * WDST], BF16,
                                                tag="xtl", name="xtl_sb")
                            nc.sync.dma_start(
                                out=xtl_sb[:, : nw4 * WDST],
                                in_=xTloc[:, w * WDST : (w + nw4) * WDST],
                            )
                            pair_tiles["xtl"] = xtl_sb
                        xr_ps = psM.tile([WDST, P], F32, tag="xr")
                        nc.tensor.matmul(
                            out=xr_ps[:],
                            lhsT=pair_tiles["xtl"][
                                :, (w % 4) * WDST : (w % 4 + 1) * WDST],
                            rhs=Wr_s[:], start=True, stop=True)
                        xw = xw_tiles[w % 2]
                        nc.vector.tensor_copy(xw[0:WDST, :], xr_ps[:])
                        out12 = psO.tile([WDST, 132], F32, tag="o12")
                        wtiles[w] = dict(xgT_sb=pair_tiles["xgT"],
                                         zrhs_sb=pair_tiles["zrhs"],
                                         sel_sb=pair_tiles["sel"],
                                         xw=xw, out12=out12,
                                         zoff=(w % 2) * KT * P,
                                         soff=(w % 2) * KT * WDST)

                    wt = wtiles[w]
                    zo, so = wt["zoff"], wt["soff"]
                    S = MW * P
                    # z feature-major: Wl.T@xgT + [xr|We|brv].T@zrhs
                    zq = psZ.tile([P, 4 * P], F32, tag="zq")
                    nc.tensor.matmul(
                        out=zq[:, :S],
                        lhsT=Wlb_s[:],
                        rhs=wt["xgT_sb"][:, zo + j0 * P : zo + j0 * P + S],
                        start=True, stop=False,
                    )
                    nc.tensor.matmul(
                        out=zq[:, :S],
                        lhsT=wt["xw"][:],
                        rhs=wt["zrhs_sb"][0:KMIX,
                                          zo + j0 * P : zo + j0 * P + S],
                        start=False, stop=True,
                    )
                    # am = |att| * leaky_relu(z)  (per-partition scale)
                    am = epool.tile([P, 4 * P], EDGE_DT, tag="am")
                    nc.scalar.activation(am[:, :S], zq[:, :S], AF.Prelu,
                                         scale=attc_s[:], alpha=NEG)
                    # value projection vq = xg@Wl, edge-major
                    vq = psV.tile([P, 4 * P], F32, tag="vq")
                    for u in range(MW):
                        j = j0 + u
                        nc.tensor.matmul(
                            out=vq[:, u * P : (u + 1) * P],
                            lhsT=wt["xgT_sb"][:, zo + j * P
                                              : zo + (j + 1) * P],
                            rhs=Wlb_s[:],
                            start=(u == 0),
                            stop=(u == MW - 1),
                        )
                    comb = epool.tile([P, 4 * 132], EDGE_DT, tag="comb")
                    gstate[g] = dict(w=w, mi=mi, j0=j0, MW=MW, vq=vq, am=am,
                                     comb=comb, sel_sb=wt["sel_sb"],
                                     out12=wt["out12"], soff=so)

                # epilogues before the next out12-start (shared PSUM bank)
                while pending_epi:
                    emit_epilogue(pending_epi.pop(0))
                if g >= 1 and g - 1 < G:
                    stage_alpha(g - 1)
                if g >= 2:
                    stage_out12(g - 2)
                    del gstate[g - 2]
            while pending_epi:
                emit_epilogue(pending_epi.pop(0))
            emit_mlp_batch()

    nc.compile()
    return nc


# ----------------------------------------------------------------------------
# Host-side preprocessing
# ----------------------------------------------------------------------------

def _preprocess(edge_index, edge_attr, ncores, nloc, nwin):
    """Route edges per core into gather-ready slot arrays.

    Node->window assignment is degree-balanced (LPT) so every window has a
    near-equal edge count, minimizing the uniform KT tile budget. Slot s of
    window w: tile j = s // 128, edge lane q = s % 128. Returns per-core
    dicts with src_slot (for the per-layer host halo gather), one-hot
    sel, the combined fp8 z-rhs slab [selT | eaT | ones], and the node
    permutation, plus kt.
    """
    src = np.ascontiguousarray(edge_index[0]).astype(np.int64)
    dst = np.ascontiguousarray(edge_index[1]).astype(np.int64)
    n = nloc * ncores
    ea = np.ascontiguousarray(edge_attr, dtype=np.float32)

    deg = np.bincount(dst, minlength=n).astype(np.float32)
    order = np.argsort(dst, kind="stable")
    dst_s = dst[order]
    src_s = src[order]
    ea_s = ea[order]
    cs = np.concatenate(
        [np.zeros((1, ea.shape[1]), np.float64), np.cumsum(ea_s, 0, dtype=np.float64)]
    )
    starts = np.searchsorted(dst_s, np.arange(n))
    ends = np.searchsorted(dst_s, np.arange(n) + 1)
    loop_attr = ((cs[ends] - cs[starts]) / np.maximum(deg, 1.0)[:, None]).astype(
        np.float32
    )

    import heapq

    cores = []
    maxcnt = 0
    for c in range(ncores):
        base = c * nloc
        # edges of this core (dst-local), self-loops appended
        lo, hi = starts[base], ends[base + nloc - 1]
        s2 = np.concatenate([src_s[lo:hi], np.arange(base, base + nloc)])
        dl = np.concatenate([dst_s[lo:hi], np.arange(base, base + nloc)]) - base
        e2 = np.concatenate([ea_s[lo:hi], loop_attr[base : base + nloc]], 0)

        # LPT balance: assign local nodes (weight = deg+1) to nwin windows
        w_of = np.empty(nloc, np.int64)
        pos_of = np.empty(nloc, np.int64)
        wdeg = (deg[base : base + nloc] + 1.0).astype(np.int64)
        heap = [(0, w, 0) for w in range(nwin)]  # (total, window, fill)
        heapq.heapify(heap)
        for node in np.argsort(-wdeg):
            tot, w, fill = heapq.heappop(heap)
            w_of[node] = w
            pos_of[node] = fill
            fill += 1
            tot += int(wdeg[node])
            if fill < WDST:
                heapq.heappush(heap, (tot, w, fill))
            else:
                heapq.heappush(heap, (1 << 60, w, fill))
        we = w_of[dl]
        pe_ = pos_of[dl]
        o = np.argsort(we, kind="stable")
        s2, e2, we, pe_ = s2[o], e2[o], we[o], pe_[o]
        wstart = np.searchsorted(we, np.arange(nwin))
        wend = np.searchsorted(we, np.arange(nwin) + 1)
        cnts = wend - wstart
        maxcnt = max(maxcnt, int(cnts.max()))
        # node permutation: slot w*WDST+pos holds local node id
        nl_flat = np.zeros(nwin * WDST, np.int64)
        nl_flat[w_of * WDST + pos_of] = np.arange(nloc)
        used = np.zeros(nwin * WDST, bool)
        used[w_of * WDST + pos_of] = True
        cores.append((s2, e2, pe_, wstart, cnts, nl_flat, used))

    kt = -(-maxcnt // P)
    S = kt * P

    data = []
    for (s2, e2, pe_, wstart, cnts, nl_flat, used) in cores:
        nslot = nwin * S
        src_slot = np.zeros(nslot, np.int64)
        dstw_slot = np.full(nslot, -1, np.int64)
        ea_slot = np.zeros((nslot, CDIM), np.float32)
        idx = np.concatenate([np.arange(cnts[w]) + w * S for w in range(nwin)])
        src_slot[idx] = s2
        dstw_slot[idx] = pe_
        ea_slot[idx] = e2

        dw = dstw_slot.reshape(nwin, kt, P)  # [w, j, q]
        # sel global 2D [q, w*kt*WDST + j*WDST + p]
        sel = (dw[:, :, :, None] == np.arange(WDST)[None, None, None, :])
        sel = np.ascontiguousarray(
            sel.transpose(2, 0, 1, 3).reshape(P, nwin * kt * WDST)
        ).astype(NP_SEL)
        # zrhs global 2D [0:64 selT | 64:96 eaT | 96 ones, w*kt*128 + j*128+q]
        zrhs = np.zeros((P, nwin * kt * P), np.float32)
        selT = (dw[:, :, None, :] == np.arange(WDST)[None, None, :, None])
        zrhs[0:WDST, :] = selT.transpose(2, 0, 1, 3).reshape(WDST, nwin * kt * P)
        zrhs[WDST : WDST + CDIM, :] = ea_slot.T
        zrhs[WDST + CDIM, :] = 1.0
        data.append(dict(src_slot=src_slot, seld=sel,
                         zrhsd=zrhs.astype(NPBF16),
                         nl_flat=nl_flat, used=used))
    return data, kt


def _layer_weight_maps(inputs, layer, att):
    """Shared (same for all cores) weight/const arrays for one layer."""
    i = layer
    attf = att[i].reshape(-1).astype(np.float32)  # [128], f = h*32+c
    sgn = np.zeros((P, H), np.float32)
    for h in range(H):
        sgn[h * CDIM : (h + 1) * CDIM, h] = np.sign(
            attf[h * CDIM : (h + 1) * CDIM]
        )
    webrv = np.concatenate(
        [
            np.asarray(inputs["We"][i], np.float32),           # [32,128]
            (np.asarray(inputs["br"][i]) + np.asarray(inputs["bl"][i]))
            .reshape(1, P)
            .astype(np.float32),
        ],
        axis=0,
    )
    m = dict(
        Wl_b=np.ascontiguousarray(inputs["Wl"][i]).astype(NP_EDGE),
        Wr=np.ascontiguousarray(inputs["Wr"][i]).astype(NPBF16),
        w1=np.ascontiguousarray(inputs["w1"][i]).astype(NPF32),
        w2=np.ascontiguousarray(inputs["w2"][i]).astype(NPF32),
        webrv=webrv.astype(NPBF16),
        attcol=np.abs(attf).reshape(P, 1).astype(NPF32),
        sgn4=sgn.astype(NPBF16),
        i64f=np.eye(WDST, dtype=NPF32),
        b1c=np.asarray(inputs["b1"][i]).reshape(P, 1).astype(NPF32),
        b2c=np.asarray(inputs["b2"][i]).reshape(P, 1).astype(NPF32),
        # gat bias + bl (bl rides the normalized softmax weights, sum ~= 1)
        bgc=(np.asarray(inputs["bias"][i]) + np.asarray(inputs["bl"][i]))
        .reshape(P, 1)
        .astype(NPF32),
    )
    return m


_NC_CACHE = {}


def kernel(**inputs):
    nodes = np.asarray(inputs["nodes"], dtype=np.float32)
    edge_index = np.asarray(inputs["edge_index"])
    edge_attr = np.asarray(inputs["edge_attr"], dtype=np.float32)

    n, d = nodes.shape
    assert (n, d) == (N, D)
    nloc = n // NCORES
    nwin = -(-nloc // WDST)

    data, kt = _preprocess(edge_index, edge_attr, NCORES, nloc, nwin)

    key = (nwin, kt, NCORES)
    if key not in _NC_CACHE:
        _NC_CACHE[key] = build_layer_nc(dict(nwin=nwin, kt=kt, ncores=NCORES))
    nc = _NC_CACHE[key]

    x_curr = np.ascontiguousarray(nodes.T)  # [128, n] f32

    for layer in range(L):
        wmap = _layer_weight_maps(inputs, layer, np.asarray(inputs["att"]))
        xce = x_curr.astype(NP_EDGE)
        in_maps = []
        for c in range(NCORES):
            base = c * nloc
            xTloc = x_curr[:, base + data[c]["nl_flat"]].copy()
            xTloc[:, ~data[c]["used"]] = 0.0
            m = dict(wmap)
            m["xgT"] = np.ascontiguousarray(xce[:, data[c]["src_slot"]])
            m["xTloc"] = xTloc.astype(NPBF16)
            m["seld"] = data[c]["seld"]
            m["zrhsd"] = data[c]["zrhsd"]
            in_maps.append(m)
        res = run_bass_kernel_spmd(
            nc, in_maps, core_ids=list(range(NCORES)), trace=TRACE
        )
        if res.exec_time_ns is not None:
            LAST_EXEC_NS.append(res.exec_time_ns)
        if TRACE:
            LAST_RESULTS.append(res)
        outs = res.results
        x_next = np.zeros((P, n), NPF32)
        for c in range(NCORES):
            xo = outs[c]["xoutT"]
            u = data[c]["used"]
            x_next[:, c * nloc + data[c]["nl_flat"][u]] = xo[:, u]
        x_curr = x_next

    return np.ascontiguousarray(x_curr.T.astype(np.float32))


# revision 11
# speedup vs baseline: 2.2001x; 1.2005x over previous
"""Bass/Trainium2 kernel for nn_BlockGNN (2-layer GATv2 + MLP) on 8 NeuronCores.

Known-good 871us configuration (run2): 128-dst windows, feature-major z,
ACT Prelu |att| trick, per-tile alpha mini-matmuls, per-window software
pipeline, f32 MLP.
"""

import os
import sys
import time

import numpy as np

os.environ.setdefault("MYCRO_LOCAL_CACHE", "1")

for _p in ("/opt/trn_rl_repo",):
    if os.path.isdir(_p) and _p not in sys.path:
        sys.path.append(_p)

import concourse.bass as bass
import concourse.bacc as bacc
import concourse.mybir as mybir
import concourse.tile as tile
from concourse.bass import AP
from concourse.bass_utils import run_bass_kernel_spmd

F32 = mybir.dt.float32
F32R = mybir.dt.float32r
BF16 = mybir.dt.bfloat16
FP8 = mybir.dt.float8e4

NPF32 = np.float32
NPBF16 = mybir.dt.np(BF16)
NPFP8 = mybir.dt.np(FP8)

N, E, D, H, CDIM, L = 50000, 800000, 128, 4, 32, 2
P = 128
NCORES = 8
NEG = 0.2

EDGE_DT = BF16
NP_EDGE = NPBF16
SEL_DT = FP8
NP_SEL = NPFP8
MLP_DT = F32
NP_MLP = NPF32
MLPB = 2
TRACE = bool(int(os.environ.get("KTRACE", "0")))

LAST_EXEC_NS = []
LAST_RESULTS = []


def _install_ntff_hook():
    try:
        import antenv.axon_hooks  # noqa: F401
        return
    except ImportError:
        pass
    import contextlib
    import ctypes
    import types

    try:
        import antenv
    except ImportError:
        return
    so_path = "/opt/axon/libaxon_pjrt.so"
    if not os.path.exists(so_path):
        return
    lib = ctypes.CDLL(so_path)
    if not hasattr(lib, "axon_start_nrt_profile"):
        return
    lib.axon_start_nrt_profile.argtypes = [
        ctypes.POINTER(ctypes.c_int64),
        ctypes.c_size_t,
    ]
    lib.axon_start_nrt_profile.restype = ctypes.c_int64
    lib.axon_stop_nrt_profile.argtypes = [ctypes.c_char_p]
    lib.axon_stop_nrt_profile.restype = ctypes.c_int64

    @contextlib.contextmanager
    def _hook(output_dir, device_ids):
        import jax

        jax.devices()
        if device_ids:
            ids = (ctypes.c_int64 * len(device_ids))(*device_ids)
            rc = lib.axon_start_nrt_profile(ids, len(device_ids))
        else:
            rc = lib.axon_start_nrt_profile(None, 0)
        if rc != 0:
            raise RuntimeError(f"axon_start_nrt_profile rc={rc}")
        try:
            yield
        finally:
            n = lib.axon_stop_nrt_profile(str(output_dir).encode())
            print(f"ntff profile: {n} file(s) -> {output_dir}", file=sys.stderr)

    mod = types.ModuleType("antenv.axon_hooks")
    _state = {"hook": _hook}
    mod.get_axon_ntff_profile_hook = lambda: _state["hook"]
    mod.set_axon_ntff_profile_hook = lambda h: _state.update(hook=h)
    sys.modules["antenv.axon_hooks"] = mod
    antenv.axon_hooks = mod


if TRACE:
    _install_ntff_hook()


def _bcast_last(ap: AP, n: int) -> AP:
    return AP(ap.tensor, ap.offset, [list(p) for p in ap.ap] + [[0, n]])


def build_layer_nc(cfg, enable_asserts=False):
    NWIN, KT = cfg["nwin"], cfg["kt"]
    NLOCP = NWIN * P
    ESLOT = NWIN * KT * P
    assert KT % 2 == 0
    MACROS = []
    j0 = 0
    while j0 < KT:
        wdt = 4 if KT - j0 >= 4 else KT - j0
        MACROS.append((j0, wdt))
        j0 += wdt
    NMAC = len(MACROS)
    assert NMAC >= 3

    nc = bacc.Bacc(
        "TRN2",
        target_bir_lowering=False,
        debug=False,
        enable_asserts=enable_asserts,
        num_devices=cfg.get("ncores", NCORES),
    )

    xgT = nc.dram_tensor("xgT", [P, ESLOT], EDGE_DT, kind="ExternalInput").ap()
    xTloc = nc.dram_tensor("xTloc", [P, NLOCP], BF16, kind="ExternalInput").ap()
    Wl_b = nc.dram_tensor("Wl_b", [P, P], EDGE_DT, kind="ExternalInput").ap()
    Wr = nc.dram_tensor("Wr", [P, P], BF16, kind="ExternalInput").ap()
    w1 = nc.dram_tensor("w1", [P, P], F32R, kind="ExternalInput").ap()
    w2 = nc.dram_tensor("w2", [P, P], F32R, kind="ExternalInput").ap()
    We_b = nc.dram_tensor("We_b", [CDIM, P], EDGE_DT, kind="ExternalInput").ap()
    brv = nc.dram_tensor("brv", [1, P], BF16, kind="ExternalInput").ap()
    ones1 = nc.dram_tensor("ones1", [1, P], BF16, kind="ExternalInput").ap()
    attcol = nc.dram_tensor("attcol", [P, 1], F32, kind="ExternalInput").ap()
    sgn4 = nc.dram_tensor("sgn4", [P, 4], BF16, kind="ExternalInput").ap()
    i128f = nc.dram_tensor("i128f", [P, P], F32, kind="ExternalInput").ap()
    b1c = nc.dram_tensor("b1c", [P, 1], F32, kind="ExternalInput").ap()
    b2c = nc.dram_tensor("b2c", [P, 1], F32, kind="ExternalInput").ap()
    bgc = nc.dram_tensor("bgc", [P, 1], F32, kind="ExternalInput").ap()
    eaT = nc.dram_tensor("eaT", [CDIM, ESLOT], EDGE_DT, kind="ExternalInput").ap()
    seld = nc.dram_tensor("seld", [NWIN, P, KT * P], SEL_DT, kind="ExternalInput").ap()
    selTd = nc.dram_tensor("selTd", [NWIN, P, KT * P], SEL_DT, kind="ExternalInput").ap()
    xoutT = nc.dram_tensor("xoutT", [P, NLOCP], F32, kind="ExternalOutput").ap()

    AF = mybir.ActivationFunctionType
    OP = mybir.AluOpType

    with tile.TileContext(nc) as tc:
        with (
            tc.tile_pool(name="const", bufs=1) as cpool,
            tc.tile_pool(name="win", bufs=2) as wpool,
            tc.tile_pool(name="edge", bufs=3) as epool,
            tc.tile_pool(name="psZ", bufs=2, space="PSUM") as psZ,
            tc.tile_pool(name="psV", bufs=2, space="PSUM") as psV,
            tc.tile_pool(name="psM", bufs=1, space="PSUM") as psM,
            tc.tile_pool(name="psO", bufs=1, space="PSUM") as psO,
            tc.tile_pool(name="psE", bufs=1, space="PSUM") as psE,
        ):
            def cload(ap, shape, dt, tag):
                t = cpool.tile(shape, dt, tag=tag)
                nc.sync.dma_start(out=t[:], in_=ap)
                return t

            Wlb_s = cload(Wl_b, [P, P], EDGE_DT, tag="Wlb_s")
            Wr_s = cload(Wr, [P, P], BF16, tag="Wr_s")
            w1_s = cload(w1, [P, P], F32R, tag="w1_s")
            w2_s = cload(w2, [P, P], F32R, tag="w2_s")
            We_s = cload(We_b, [CDIM, P], EDGE_DT, tag="We_s")
            brv_s = cload(brv, [1, P], BF16, tag="brv_s")
            ones_s = cload(ones1, [1, P], BF16, tag="ones_s")
            attc_s = cload(attcol, [P, 1], F32, tag="attc_s")
            sgn_s = cload(sgn4, [P, 4], BF16, tag="sgn_s")
            i128f_s = cload(i128f, [P, P], F32, tag="i128f_s")
            b1c_s = cload(b1c, [P, 1], F32, tag="b1c_s")
            b2c_s = cload(b2c, [P, 1], F32, tag="b2c_s")
            bgc_s = cload(bgc, [P, 1], F32, tag="bgc_s")

            mlp_ctx = {"gTall": None, "w0": None, "nw": 0}

            def emit_mlp_batch():
                gTall, w0, nw = mlp_ctx["gTall"], mlp_ctx["w0"], mlp_ctx["nw"]
                if gTall is None:
                    return
                mlp_ctx["gTall"] = None
                S = nw * P
                gTb = wpool.tile([P, MLPB * P], F32R, tag="gTb")
                nc.scalar.activation(gTb[:, :S], gTall[:, :S], AF.Identity,
                                     bias=bgc_s[:])
                y1_ps = psE.tile([P, MLPB * P], F32, tag="epi", name="y1_ps")
                nc.tensor.matmul(out=y1_ps[:, :S], lhsT=w1_s[:],
                                 rhs=gTb[:, :S], start=True, stop=True)
                y1s = wpool.tile([P, MLPB * P], F32R, tag="y1s")
                nc.scalar.activation(y1s[:, :S], y1_ps[:, :S], AF.Relu,
                                     bias=b1c_s[:])
                y2_ps = psE.tile([P, MLPB * P], F32, tag="epi", name="y2_ps")
                nc.tensor.matmul(out=y2_ps[:, :S], lhsT=w2_s[:],
                                 rhs=y1s[:, :S], start=True, stop=True)
                xo = wpool.tile([P, MLPB * P], F32, tag="xo")
                nc.scalar.activation(xo[:, :S], y2_ps[:, :S], AF.Identity,
                                     bias=b2c_s[:])
                nc.sync.dma_start(out=xoutT[:, w0 * P : w0 * P + S],
                                  in_=xo[:, :S])

            pending_epilogue = [None]

            def emit_epilogue():
                fn = pending_epilogue[0]
                if fn is not None:
                    pending_epilogue[0] = None
                    fn()

            for w in range(NWIN):
                xgT_sb = wpool.tile([P, KT * P], EDGE_DT, tag="xgT")
                nc.sync.dma_start(
                    out=xgT_sb[:], in_=xgT[:, w * KT * P : (w + 1) * KT * P]
                )
                eaT_sb = wpool.tile([CDIM, KT * P], EDGE_DT, tag="ea")
                nc.sync.dma_start(
                    out=eaT_sb[:], in_=eaT[:, w * KT * P : (w + 1) * KT * P]
                )
                sel_sb = wpool.tile([P, KT * P], SEL_DT, tag="sel")
                nc.sync.dma_start(out=sel_sb[:], in_=seld[w])
                selT_sb = wpool.tile([P, KT * P], SEL_DT, tag="selT")
                nc.sync.dma_start(out=selT_sb[:], in_=selTd[w])

                xtl_sb = wpool.tile([P, P], BF16, tag="xtl")
                nc.sync.dma_start(out=xtl_sb[:], in_=xTloc[:, w * P : (w + 1) * P])
                xr_ps = psE.tile([P, P], F32, tag="xr")
                nc.tensor.matmul(out=xr_ps[:], lhsT=xtl_sb[:], rhs=Wr_s[:],
                                 start=True, stop=False)
                nc.tensor.matmul(out=xr_ps[:], lhsT=ones_s[:], rhs=brv_s[:],
                                 start=False, stop=True)
                xr_sb = wpool.tile([P, P], EDGE_DT, tag="xrs")
                nc.scalar.activation(xr_sb[:], xr_ps[:], AF.Copy)

                out12 = psO.tile([P, 132], F32, tag="o12")

                state = [None] * NMAC

                def stage_alpha(mi):
                    j0, MW, vq, am, comb = state[mi]
                    mini = psM.tile([P, 16], F32, tag="mini")
                    for u in range(MW):
                        nc.tensor.matmul(
                            out=mini[:, u * 4 : (u + 1) * 4],
                            lhsT=am[:, u * P : (u + 1) * P],
                            rhs=sgn_s[:],
                            start=(u == 0),
                            stop=(u == MW - 1),
                        )
                    comb_v = comb[:, : MW * 132].rearrange("p (b f) -> p b f",
                                                           f=132)
                    nc.scalar.activation(
                        comb_v[:, :, P : P + 4],
                        mini[:, : MW * 4].rearrange("p (b h) -> p b h", h=4),
                        AF.Exp,
                    )
                    nc.vector.tensor_tensor(
                        comb_v[:, :, 0:P].rearrange("p b (h c) -> p b h c", c=CDIM),
                        vq[:, : MW * P].rearrange("p (b h c) -> p b h c",
                                                  b=MW, c=CDIM),
                        _bcast_last(comb_v[:, :, P : P + 4], CDIM),
                        op=OP.mult,
                    )

                def stage_out12(mi):
                    j0, MW, vq, am, comb = state[mi]
                    for u in range(MW):
                        j = j0 + u
                        nc.tensor.matmul(
                            out=out12[:],
                            lhsT=sel_sb[:, j * P : (j + 1) * P],
                            rhs=comb[:, u * 132 : (u + 1) * 132],
                            start=(mi == 0 and u == 0),
                            stop=(mi == NMAC - 1 and u == MW - 1),
                        )

                for mi, (j0, MW) in enumerate(MACROS):
                    S = MW * P
                    zq = psZ.tile([P, 4 * P], F32, tag="zq")
                    nc.tensor.matmul(
                        out=zq[:, :S],
                        lhsT=We_s[:],
                        rhs=eaT_sb[:, j0 * P : j0 * P + S],
                        start=True, stop=False,
                    )
                    nc.tensor.matmul(
                        out=zq[:, :S],
                        lhsT=Wlb_s[:],
                        rhs=xgT_sb[:, j0 * P : j0 * P + S],
                        start=False, stop=False,
                    )
                    nc.tensor.matmul(
                        out=zq[:, :S],
                        lhsT=xr_sb[:],
                        rhs=selT_sb[:, j0 * P : j0 * P + S],
                        start=False, stop=True,
                    )
                    am = epool.tile([P, 4 * P], EDGE_DT, tag="am")
                    nc.scalar.activation(am[:, :S], zq[:, :S], AF.Prelu,
                                         scale=attc_s[:], alpha=NEG)
                    vq = psV.tile([P, 4 * P], F32, tag="vq")
                    for u in range(MW):
                        j = j0 + u
                        nc.tensor.matmul(
                            out=vq[:, u * P : (u + 1) * P],
                            lhsT=xgT_sb[:, j * P : (j + 1) * P],
                            rhs=Wlb_s[:],
                            start=(u == 0),
                            stop=(u == MW - 1),
                        )
                    comb = epool.tile([P, 4 * 132], EDGE_DT, tag="comb")
                    state[mi] = (j0, MW, vq, am, comb)

                    if mi == 0:
                        emit_epilogue()
                    if mi >= 1:
                        stage_alpha(mi - 1)
                    if mi >= 2:
                        stage_out12(mi - 2)

                stage_alpha(NMAC - 1)
                stage_out12(NMAC - 2)
                stage_out12(NMAC - 1)

                def make_epilogue(w, out12):
                    def epi():
                        de = wpool.tile([P, 4], F32, tag="de")
                        nc.vector.tensor_scalar(de[:], out12[:, P : P + 4],
                                                1e-16, None, OP.add)
                        rc = wpool.tile([P, 4], F32, tag="rc")
                        nc.vector.reciprocal(rc[:], de[:])
                        gat = wpool.tile([P, P], F32, tag="gat")
                        for h in range(H):
                            nc.vector.tensor_scalar(
                                gat[:, h * CDIM : (h + 1) * CDIM],
                                out12[:, h * CDIM : (h + 1) * CDIM],
                                rc[:, h : h + 1],
                                None,
                                OP.mult,
                            )
                        if mlp_ctx["gTall"] is None:
                            mlp_ctx["gTall"] = psE.tile(
                                [P, MLPB * P], F32, tag="epi", name="gTall")
                            mlp_ctx["w0"] = w
                            mlp_ctx["nw"] = 0
                        k = mlp_ctx["nw"]
                        nc.tensor.transpose(
                            mlp_ctx["gTall"][:, k * P : (k + 1) * P],
                            gat[:], i128f_s[:])
                        mlp_ctx["nw"] = k + 1
                        if mlp_ctx["nw"] == MLPB:
                            emit_mlp_batch()
                    return epi

                pending_epilogue[0] = make_epilogue(w, out12)

            emit_epilogue()
            emit_mlp_batch()

    nc.compile()
    return nc


def _preprocess(edge_index, edge_attr, ncores, nloc, nwin):
    src = np.ascontiguousarray(edge_index[0]).astype(np.int64)
    dst = np.ascontiguousarray(edge_index[1]).astype(np.int64)
    n = nloc * ncores
    ea = np.ascontiguousarray(edge_attr, dtype=np.float32)

    deg = np.bincount(dst, minlength=n).astype(np.float32)
    order = np.argsort(dst, kind="stable")
    dst_s = dst[order]
    src_s = src[order]
    ea_s = ea[order]
    cs = np.concatenate(
        [np.zeros((1, ea.shape[1]), np.float64), np.cumsum(ea_s, 0, dtype=np.float64)]
    )
    starts = np.searchsorted(dst_s, np.arange(n))
    ends = np.searchsorted(dst_s, np.arange(n) + 1)
    loop_attr = ((cs[ends] - cs[starts]) / np.maximum(deg, 1.0)[:, None]).astype(
        np.float32
    )

    import heapq

    cores = []
    maxcnt = 0
    for c in range(ncores):
        base = c * nloc
        lo, hi = starts[base], ends[base + nloc - 1]
        s2 = np.concatenate([src_s[lo:hi], np.arange(base, base + nloc)])
        dl = np.concatenate([dst_s[lo:hi], np.arange(base, base + nloc)]) - base
        e2 = np.concatenate([ea_s[lo:hi], loop_attr[base : base + nloc]], 0)

        w_of = np.empty(nloc, np.int64)
        pos_of = np.empty(nloc, np.int64)
        wdeg = (deg[base : base + nloc] + 1.0).astype(np.int64)
        heap = [(0, w, 0) for w in range(nwin)]
        heapq.heapify(heap)
        for node in np.argsort(-wdeg):
            tot, w, fill = heapq.heappop(heap)
            w_of[node] = w
            pos_of[node] = fill
            fill += 1
            tot += int(wdeg[node])
            if fill < P:
                heapq.heappush(heap, (tot, w, fill))
            else:
                heapq.heappush(heap, (1 << 60, w, fill))
        we = w_of[dl]
        pe_ = pos_of[dl]
        o = np.argsort(we, kind="stable")
        s2, e2, we, pe_ = s2[o], e2[o], we[o], pe_[o]
        wstart = np.searchsorted(we, np.arange(nwin))
        wend = np.searchsorted(we, np.arange(nwin) + 1)
        cnts = wend - wstart
        maxcnt = max(maxcnt, int(cnts.max()))
        nl_flat = np.zeros(nwin * P, np.int64)
        nl_flat[w_of * P + pos_of] = np.arange(nloc)
        used = np.zeros(nwin * P, bool)
        used[w_of * P + pos_of] = True
        cores.append((s2, e2, pe_, wstart, cnts, nl_flat, used))

    kt = -(-maxcnt // P)
    if kt % 2:
        kt += 1
    S = kt * P

    data = []
    for (s2, e2, pe_, wstart, cnts, nl_flat, used) in cores:
        nslot = nwin * S
        src_slot = np.zeros(nslot, np.int64)
        dstw_slot = np.full(nslot, -1, np.int64)
        ea_slot = np.zeros((nslot, CDIM), np.float32)
        idx = np.concatenate([np.arange(cnts[w]) + w * S for w in range(nwin)])
        src_slot[idx] = s2
        dstw_slot[idx] = pe_
        ea_slot[idx] = e2

        dw = dstw_slot.reshape(nwin, kt, P)
        sel = (dw[:, :, :, None] == np.arange(P)[None, None, None, :])
        sel = sel.transpose(0, 2, 1, 3).reshape(nwin, P, kt * P).astype(NP_SEL)
        selT = (dw[:, :, None, :] == np.arange(P)[None, None, :, None])
        selT = selT.transpose(0, 2, 1, 3).reshape(nwin, P, kt * P).astype(NP_SEL)
        eaT = np.ascontiguousarray(ea_slot.T).astype(NP_EDGE)
        data.append(dict(src_slot=src_slot, seld=sel, selTd=selT, eaT=eaT,
                         nl_flat=nl_flat, used=used))
    return data, kt


def _layer_weight_maps(inputs, layer, att):
    i = layer
    attf = att[i].reshape(-1).astype(np.float32)
    sgn = np.zeros((P, H), np.float32)
    for h in range(H):
        sgn[h * CDIM : (h + 1) * CDIM, h] = np.sign(
            attf[h * CDIM : (h + 1) * CDIM]
        )
    m = dict(
        Wl_b=np.ascontiguousarray(inputs["Wl"][i]).astype(NP_EDGE),
        Wr=np.ascontiguousarray(inputs["Wr"][i]).astype(NPBF16),
        w1=np.ascontiguousarray(inputs["w1"][i]).astype(NP_MLP),
        w2=np.ascontiguousarray(inputs["w2"][i]).astype(NP_MLP),
        We_b=np.ascontiguousarray(inputs["We"][i]).astype(NP_EDGE),
        brv=(np.asarray(inputs["br"][i]) + np.asarray(inputs["bl"][i]))
        .reshape(1, P)
        .astype(NPBF16),
        ones1=np.ones((1, P), NPBF16),
        attcol=np.abs(attf).reshape(P, 1).astype(NPF32),
        sgn4=sgn.astype(NPBF16),
        i128f=np.eye(P, dtype=NPF32),
        b1c=np.asarray(inputs["b1"][i]).reshape(P, 1).astype(NPF32),
        b2c=np.asarray(inputs["b2"][i]).reshape(P, 1).astype(NPF32),
        bgc=(np.asarray(inputs["bias"][i]) + np.asarray(inputs["bl"][i]))
        .reshape(P, 1)
        .astype(NPF32),
    )
    return m


_NC_CACHE = {}


def kernel(**inputs):
    nodes = np.asarray(inputs["nodes"], dtype=np.float32)
    edge_index = np.asarray(inputs["edge_index"])
    edge_attr = np.asarray(inputs["edge_attr"], dtype=np.float32)

    n, d = nodes.shape
    assert (n, d) == (N, D)
    nloc = n // NCORES
    nwin = -(-nloc // P)

    data, kt = _preprocess(edge_index, edge_attr, NCORES, nloc, nwin)

    key = (nwin, kt, NCORES)
    if key not in _NC_CACHE:
        _NC_CACHE[key] = build_layer_nc(dict(nwin=nwin, kt=kt, ncores=NCORES))
    nc = _NC_CACHE[key]

    x_curr = np.ascontiguousarray(nodes.T)

    for layer in range(L):
        wmap = _layer_weight_maps(inputs, layer, np.asarray(inputs["att"]))
        xce = x_curr.astype(NP_EDGE)
        in_maps = []
        for c in range(NCORES):
            base = c * nloc
            xTloc = x_curr[:, base + data[c]["nl_flat"]].copy()
            xTloc[:, ~data[c]["used"]] = 0.0
            m = dict(wmap)
            m["xgT"] = np.ascontiguousarray(xce[:, data[c]["src_slot"]])
            m["xTloc"] = xTloc.astype(NPBF16)
            m["seld"] = data[c]["seld"]
            m["selTd"] = data[c]["selTd"]
            m["eaT"] = data[c]["eaT"]
            in_maps.append(m)
        res = run_bass_kernel_spmd(
            nc, in_maps, core_ids=list(range(NCORES)), trace=TRACE
        )
        if res.exec_time_ns is not None:
            LAST_EXEC_NS.append(res.exec_time_ns)
        if TRACE:
            LAST_RESULTS.append(res)
        outs = res.results
        x_next = np.zeros((P, n), NPF32)
        for c in range(NCORES):
            xo = outs[c]["xoutT"]
            u = data[c]["used"]
            x_next[:, c * nloc + data[c]["nl_flat"][u]] = xo[:, u]
        x_curr = x_next

    return np.ascontiguousarray(x_curr.T.astype(np.float32))


# revision 12
# speedup vs baseline: 2.4584x; 1.1174x over previous
"""Bass/Trainium2 kernel for nn_BlockGNN (2-layer GATv2 + MLP) on 8 NeuronCores.

Known-good 871us configuration (run2): 128-dst windows, feature-major z,
ACT Prelu |att| trick, per-tile alpha mini-matmuls, per-window software
pipeline, f32 MLP.
"""

import os
import sys
import time

import numpy as np

os.environ.setdefault("MYCRO_LOCAL_CACHE", "1")

for _p in ("/opt/trn_rl_repo",):
    if os.path.isdir(_p) and _p not in sys.path:
        sys.path.append(_p)

import concourse.bass as bass
import concourse.bacc as bacc
import concourse.mybir as mybir
import concourse.tile as tile
from concourse.bass import AP
from concourse.bass_utils import run_bass_kernel_spmd

F32 = mybir.dt.float32
BF16 = mybir.dt.bfloat16
FP8 = mybir.dt.float8e4

NPF32 = np.float32
NPBF16 = mybir.dt.np(BF16)
NPFP8 = mybir.dt.np(FP8)

N, E, D, H, CDIM, L = 50000, 800000, 128, 4, 32, 2
P = 128
NCORES = 8
NEG = 0.2

EDGE_DT = BF16
NP_EDGE = NPBF16
SEL_DT = FP8
NP_SEL = NPFP8
MLP_DT = F32
NP_MLP = NPF32
TRACE = bool(int(os.environ.get("KTRACE", "0")))

LAST_EXEC_NS = []
LAST_RESULTS = []


def _install_ntff_hook():
    try:
        import antenv.axon_hooks  # noqa: F401
        return
    except ImportError:
        pass
    import contextlib
    import ctypes
    import types

    try:
        import antenv
    except ImportError:
        return
    so_path = "/opt/axon/libaxon_pjrt.so"
    if not os.path.exists(so_path):
        return
    lib = ctypes.CDLL(so_path)
    if not hasattr(lib, "axon_start_nrt_profile"):
        return
    lib.axon_start_nrt_profile.argtypes = [
        ctypes.POINTER(ctypes.c_int64),
        ctypes.c_size_t,
    ]
    lib.axon_start_nrt_profile.restype = ctypes.c_int64
    lib.axon_stop_nrt_profile.argtypes = [ctypes.c_char_p]
    lib.axon_stop_nrt_profile.restype = ctypes.c_int64

    @contextlib.contextmanager
    def _hook(output_dir, device_ids):
        import jax

        jax.devices()
        if device_ids:
            ids = (ctypes.c_int64 * len(device_ids))(*device_ids)
            rc = lib.axon_start_nrt_profile(ids, len(device_ids))
        else:
            rc = lib.axon_start_nrt_profile(None, 0)
        if rc != 0:
            raise RuntimeError(f"axon_start_nrt_profile rc={rc}")
        try:
            yield
        finally:
            n = lib.axon_stop_nrt_profile(str(output_dir).encode())
            print(f"ntff profile: {n} file(s) -> {output_dir}", file=sys.stderr)

    mod = types.ModuleType("antenv.axon_hooks")
    _state = {"hook": _hook}
    mod.get_axon_ntff_profile_hook = lambda: _state["hook"]
    mod.set_axon_ntff_profile_hook = lambda h: _state.update(hook=h)
    sys.modules["antenv.axon_hooks"] = mod
    antenv.axon_hooks = mod


if TRACE:
    _install_ntff_hook()


def _bcast_last(ap: AP, n: int) -> AP:
    return AP(ap.tensor, ap.offset, [list(p) for p in ap.ap] + [[0, n]])


def build_layer_nc(cfg, enable_asserts=False):
    NWIN, KT = cfg["nwin"], cfg["kt"]
    NLOCP = NWIN * P
    ESLOT = NWIN * KT * P
    assert KT % 2 == 0
    MACROS = []
    j0 = 0
    while j0 < KT:
        wdt = 4 if KT - j0 >= 4 else KT - j0
        MACROS.append((j0, wdt))
        j0 += wdt
    NMAC = len(MACROS)
    assert NMAC >= 3

    nc = bacc.Bacc(
        "TRN2",
        target_bir_lowering=False,
        debug=False,
        enable_asserts=enable_asserts,
        num_devices=cfg.get("ncores", NCORES),
    )

    xgT = nc.dram_tensor("xgT", [P, ESLOT], EDGE_DT, kind="ExternalInput").ap()
    xTloc = nc.dram_tensor("xTloc", [P, NLOCP], BF16, kind="ExternalInput").ap()
    Wl_b = nc.dram_tensor("Wl_b", [P, P], EDGE_DT, kind="ExternalInput").ap()
    Wr = nc.dram_tensor("Wr", [P, P], BF16, kind="ExternalInput").ap()
    w1 = nc.dram_tensor("w1", [P, P], MLP_DT, kind="ExternalInput").ap()
    w2 = nc.dram_tensor("w2", [P, P], MLP_DT, kind="ExternalInput").ap()
    We_b = nc.dram_tensor("We_b", [CDIM, P], EDGE_DT, kind="ExternalInput").ap()
    brv = nc.dram_tensor("brv", [1, P], BF16, kind="ExternalInput").ap()
    ones1 = nc.dram_tensor("ones1", [1, P], BF16, kind="ExternalInput").ap()
    attcol = nc.dram_tensor("attcol", [P, 1], F32, kind="ExternalInput").ap()
    sgn4 = nc.dram_tensor("sgn4", [P, 4], BF16, kind="ExternalInput").ap()
    i128f = nc.dram_tensor("i128f", [P, P], F32, kind="ExternalInput").ap()
    b1c = nc.dram_tensor("b1c", [P, 1], F32, kind="ExternalInput").ap()
    b2c = nc.dram_tensor("b2c", [P, 1], F32, kind="ExternalInput").ap()
    bgc = nc.dram_tensor("bgc", [P, 1], F32, kind="ExternalInput").ap()
    eaT = nc.dram_tensor("eaT", [CDIM, ESLOT], EDGE_DT, kind="ExternalInput").ap()
    seld = nc.dram_tensor("seld", [NWIN, P, KT * P], SEL_DT, kind="ExternalInput").ap()
    selTd = nc.dram_tensor("selTd", [NWIN, P, KT * P], SEL_DT, kind="ExternalInput").ap()
    xoutT = nc.dram_tensor("xoutT", [P, NLOCP], F32, kind="ExternalOutput").ap()

    AF = mybir.ActivationFunctionType
    OP = mybir.AluOpType

    with tile.TileContext(nc) as tc:
        with (
            tc.tile_pool(name="const", bufs=1) as cpool,
            tc.tile_pool(name="win", bufs=2) as wpool,
            tc.tile_pool(name="edge", bufs=3) as epool,
            tc.tile_pool(name="psZ", bufs=2, space="PSUM") as psZ,
            tc.tile_pool(name="psV", bufs=2, space="PSUM") as psV,
            tc.tile_pool(name="psM", bufs=1, space="PSUM") as psM,
            tc.tile_pool(name="psO", bufs=1, space="PSUM") as psO,
            tc.tile_pool(name="psE", bufs=1, space="PSUM") as psE,
        ):
            def cload(ap, shape, dt, tag):
                t = cpool.tile(shape, dt, tag=tag)
                nc.sync.dma_start(out=t[:], in_=ap)
                return t

            Wlb_s = cload(Wl_b, [P, P], EDGE_DT, tag="Wlb_s")
            Wr_s = cload(Wr, [P, P], BF16, tag="Wr_s")
            w1_s = cload(w1, [P, P], MLP_DT, tag="w1_s")
            w2_s = cload(w2, [P, P], MLP_DT, tag="w2_s")
            We_s = cload(We_b, [CDIM, P], EDGE_DT, tag="We_s")
            brv_s = cload(brv, [1, P], BF16, tag="brv_s")
            ones_s = cload(ones1, [1, P], BF16, tag="ones_s")
            attc_s = cload(attcol, [P, 1], F32, tag="attc_s")
            sgn_s = cload(sgn4, [P, 4], BF16, tag="sgn_s")
            i128f_s = cload(i128f, [P, P], F32, tag="i128f_s")
            b1c_s = cload(b1c, [P, 1], F32, tag="b1c_s")
            b2c_s = cload(b2c, [P, 1], F32, tag="b2c_s")
            bgc_s = cload(bgc, [P, 1], F32, tag="bgc_s")

            pending_epilogue = [None]

            def emit_epilogue():
                fn = pending_epilogue[0]
                if fn is not None:
                    pending_epilogue[0] = None
                    fn()

            for w in range(NWIN):
                xgT_sb = wpool.tile([P, KT * P], EDGE_DT, tag="xgT")
                nc.sync.dma_start(
                    out=xgT_sb[:], in_=xgT[:, w * KT * P : (w + 1) * KT * P]
                )
                eaT_sb = wpool.tile([CDIM, KT * P], EDGE_DT, tag="ea")
                nc.sync.dma_start(
                    out=eaT_sb[:], in_=eaT[:, w * KT * P : (w + 1) * KT * P]
                )
                sel_sb = wpool.tile([P, KT * P], SEL_DT, tag="sel")
                nc.sync.dma_start(out=sel_sb[:], in_=seld[w])
                selT_sb = wpool.tile([P, KT * P], SEL_DT, tag="selT")
                nc.sync.dma_start(out=selT_sb[:], in_=selTd[w])

                xtl_sb = wpool.tile([P, P], BF16, tag="xtl")
                nc.sync.dma_start(out=xtl_sb[:], in_=xTloc[:, w * P : (w + 1) * P])
                xr_ps = psE.tile([P, P], F32, tag="xr")
                nc.tensor.matmul(out=xr_ps[:], lhsT=xtl_sb[:], rhs=Wr_s[:],
                                 start=True, stop=False)
                nc.tensor.matmul(out=xr_ps[:], lhsT=ones_s[:], rhs=brv_s[:],
                                 start=False, stop=True)
                xr_sb = wpool.tile([P, P], EDGE_DT, tag="xrs")
                nc.scalar.activation(xr_sb[:], xr_ps[:], AF.Copy)

                out12 = psO.tile([P, 132], F32, tag="o12")

                state = [None] * NMAC

                def stage_alpha(mi):
                    j0, MW, vq, am, comb = state[mi]
                    mini = psM.tile([P, 16], F32, tag="mini")
                    for u in range(MW):
                        nc.tensor.matmul(
                            out=mini[:, u * 4 : (u + 1) * 4],
                            lhsT=am[:, u * P : (u + 1) * P],
                            rhs=sgn_s[:],
                            start=(u == 0),
                            stop=(u == MW - 1),
                        )
                    comb_v = comb[:, : MW * 132].rearrange("p (b f) -> p b f",
                                                           f=132)
                    nc.scalar.activation(
                        comb_v[:, :, P : P + 4],
                        mini[:, : MW * 4].rearrange("p (b h) -> p b h", h=4),
                        AF.Exp,
                    )
                    nc.vector.tensor_tensor(
                        comb_v[:, :, 0:P].rearrange("p b (h c) -> p b h c", c=CDIM),
                        vq[:, : MW * P].rearrange("p (b h c) -> p b h c",
                                                  b=MW, c=CDIM),
                        _bcast_last(comb_v[:, :, P : P + 4], CDIM),
                        op=OP.mult,
                    )

                def stage_out12(mi):
                    j0, MW, vq, am, comb = state[mi]
                    for u in range(MW):
                        j = j0 + u
                        nc.tensor.matmul(
                            out=out12[:],
                            lhsT=sel_sb[:, j * P : (j + 1) * P],
                            rhs=comb[:, u * 132 : (u + 1) * 132],
                            start=(mi == 0 and u == 0),
                            stop=(mi == NMAC - 1 and u == MW - 1),
                        )

                for mi, (j0, MW) in enumerate(MACROS):
                    S = MW * P
                    zq = psZ.tile([P, 4 * P], F32, tag="zq")
                    nc.tensor.matmul(
                        out=zq[:, :S],
                        lhsT=We_s[:],
                        rhs=eaT_sb[:, j0 * P : j0 * P + S],
                        start=True, stop=False,
                    )
                    nc.tensor.matmul(
                        out=zq[:, :S],
                        lhsT=Wlb_s[:],
                        rhs=xgT_sb[:, j0 * P : j0 * P + S],
                        start=False, stop=False,
                    )
                    nc.tensor.matmul(
                        out=zq[:, :S],
                        lhsT=xr_sb[:],
                        rhs=selT_sb[:, j0 * P : j0 * P + S],
                        start=False, stop=True,
                    )
                    am = epool.tile([P, 4 * P], EDGE_DT, tag="am")
                    nc.scalar.activation(am[:, :S], zq[:, :S], AF.Prelu,
                                         scale=attc_s[:], alpha=NEG)
                    vq = psV.tile([P, 4 * P], F32, tag="vq")
                    for u in range(MW):
                        j = j0 + u
                        nc.tensor.matmul(
                            out=vq[:, u * P : (u + 1) * P],
                            lhsT=xgT_sb[:, j * P : (j + 1) * P],
                            rhs=Wlb_s[:],
                            start=(u == 0),
                            stop=(u == MW - 1),
                        )
                    comb = epool.tile([P, 4 * 132], EDGE_DT, tag="comb")
                    state[mi] = (j0, MW, vq, am, comb)

                    if mi == 0:
                        emit_epilogue()
                    if mi >= 1:
                        stage_alpha(mi - 1)
                    if mi >= 2:
                        stage_out12(mi - 2)

                stage_alpha(NMAC - 1)
                stage_out12(NMAC - 2)
                stage_out12(NMAC - 1)

                def make_epilogue(w, out12):
                    def epi():
                        de = wpool.tile([P, 4], F32, tag="de")
                        nc.vector.tensor_scalar(de[:], out12[:, P : P + 4],
                                                1e-16, None, OP.add)
                        rc = wpool.tile([P, 4], F32, tag="rc")
                        nc.vector.reciprocal(rc[:], de[:])
                        gat = wpool.tile([P, P], F32, tag="gat")
                        for h in range(H):
                            nc.vector.tensor_scalar(
                                gat[:, h * CDIM : (h + 1) * CDIM],
                                out12[:, h * CDIM : (h + 1) * CDIM],
                                rc[:, h : h + 1],
                                None,
                                OP.mult,
                            )
                        gatT_ps = psE.tile([P, P], F32, tag="epi")
                        nc.tensor.transpose(gatT_ps[:], gat[:], i128f_s[:])
                        gTb = wpool.tile([P, P], MLP_DT, tag="gTb")
                        nc.scalar.activation(gTb[:], gatT_ps[:], AF.Identity,
                                             bias=bgc_s[:])
                        y1_ps = psE.tile([P, P], F32, tag="epi")
                        nc.tensor.matmul(out=y1_ps[:], lhsT=w1_s[:], rhs=gTb[:],
                                         start=True, stop=True)
                        y1s = wpool.tile([P, P], MLP_DT, tag="y1s")
                        nc.scalar.activation(y1s[:], y1_ps[:], AF.Relu,
                                             bias=b1c_s[:])
                        y2_ps = psE.tile([P, P], F32, tag="epi")
                        nc.tensor.matmul(out=y2_ps[:], lhsT=w2_s[:], rhs=y1s[:],
                                         start=True, stop=True)
                        xo = wpool.tile([P, P], F32, tag="xo")
                        nc.scalar.activation(xo[:], y2_ps[:], AF.Identity,
                                             bias=b2c_s[:])
                        nc.sync.dma_start(out=xoutT[:, w * P : (w + 1) * P],
                                          in_=xo[:])
                    return epi

                pending_epilogue[0] = make_epilogue(w, out12)

            emit_epilogue()

    nc.compile()
    return nc


def _preprocess(edge_index, edge_attr, ncores, nloc, nwin):
    src = np.ascontiguousarray(edge_index[0]).astype(np.int64)
    dst = np.ascontiguousarray(edge_index[1]).astype(np.int64)
    n = nloc * ncores
    ea = np.ascontiguousarray(edge_attr, dtype=np.float32)

    deg = np.bincount(dst, minlength=n).astype(np.float32)
    order = np.argsort(dst, kind="stable")
    dst_s = dst[order]
    src_s = src[order]
    ea_s = ea[order]
    cs = np.concatenate(
        [np.zeros((1, ea.shape[1]), np.float64), np.cumsum(ea_s, 0, dtype=np.float64)]
    )
    starts = np.searchsorted(dst_s, np.arange(n))
    ends = np.searchsorted(dst_s, np.arange(n) + 1)
    loop_attr = ((cs[ends] - cs[starts]) / np.maximum(deg, 1.0)[:, None]).astype(
        np.float32
    )

    import heapq

    cores = []
    maxcnt = 0
    for c in range(ncores):
        base = c * nloc
        lo, hi = starts[base], ends[base + nloc - 1]
        s2 = np.concatenate([src_s[lo:hi], np.arange(base, base + nloc)])
        dl = np.concatenate([dst_s[lo:hi], np.arange(base, base + nloc)]) - base
        e2 = np.concatenate([ea_s[lo:hi], loop_attr[base : base + nloc]], 0)

        w_of = np.empty(nloc, np.int64)
        pos_of = np.empty(nloc, np.int64)
        wdeg = (deg[base : base + nloc] + 1.0).astype(np.int64)
        heap = [(0, w, 0) for w in range(nwin)]
        heapq.heapify(heap)
        for node in np.argsort(-wdeg):
            tot, w, fill = heapq.heappop(heap)
            w_of[node] = w
            pos_of[node] = fill
            fill += 1
            tot += int(wdeg[node])
            if fill < P:
                heapq.heappush(heap, (tot, w, fill))
            else:
                heapq.heappush(heap, (1 << 60, w, fill))
        we = w_of[dl]
        pe_ = pos_of[dl]
        o = np.argsort(we, kind="stable")
        s2, e2, we, pe_ = s2[o], e2[o], we[o], pe_[o]
        wstart = np.searchsorted(we, np.arange(nwin))
        wend = np.searchsorted(we, np.arange(nwin) + 1)
        cnts = wend - wstart
        maxcnt = max(maxcnt, int(cnts.max()))
        nl_flat = np.zeros(nwin * P, np.int64)
        nl_flat[w_of * P + pos_of] = np.arange(nloc)
        used = np.zeros(nwin * P, bool)
        used[w_of * P + pos_of] = True
        cores.append((s2, e2, pe_, wstart, cnts, nl_flat, used))

    kt = -(-maxcnt // P)
    if kt % 2:
        kt += 1
    S = kt * P

    data = []
    for (s2, e2, pe_, wstart, cnts, nl_flat, used) in cores:
        nslot = nwin * S
        src_slot = np.zeros(nslot, np.int64)
        dstw_slot = np.full(nslot, -1, np.int64)
        ea_slot = np.zeros((nslot, CDIM), np.float32)
        idx = np.concatenate([np.arange(cnts[w]) + w * S for w in range(nwin)])
        src_slot[idx] = s2
        dstw_slot[idx] = pe_
        ea_slot[idx] = e2

        dw = dstw_slot.reshape(nwin, kt, P)
        sel = (dw[:, :, :, None] == np.arange(P)[None, None, None, :])
        sel = sel.transpose(0, 2, 1, 3).reshape(nwin, P, kt * P).astype(NP_SEL)
        selT = (dw[:, :, None, :] == np.arange(P)[None, None, :, None])
        selT = selT.transpose(0, 2, 1, 3).reshape(nwin, P, kt * P).astype(NP_SEL)
        eaT = np.ascontiguousarray(ea_slot.T).astype(NP_EDGE)
        data.append(dict(src_slot=src_slot, seld=sel, selTd=selT, eaT=eaT,
                         nl_flat=nl_flat, used=used))
    return data, kt


def _layer_weight_maps(inputs, layer, att):
    i = layer
    attf = att[i].reshape(-1).astype(np.float32)
    sgn = np.zeros((P, H), np.float32)
    for h in range(H):
        sgn[h * CDIM : (h + 1) * CDIM, h] = np.sign(
            attf[h * CDIM : (h + 1) * CDIM]
        )
    m = dict(
        Wl_b=np.ascontiguousarray(inputs["Wl"][i]).astype(NP_EDGE),
        Wr=np.ascontiguousarray(inputs["Wr"][i]).astype(NPBF16),
        w1=np.ascontiguousarray(inputs["w1"][i]).astype(NP_MLP),
        w2=np.ascontiguousarray(inputs["w2"][i]).astype(NP_MLP),
        We_b=np.ascontiguousarray(inputs["We"][i]).astype(NP_EDGE),
        brv=(np.asarray(inputs["br"][i]) + np.asarray(inputs["bl"][i]))
        .reshape(1, P)
        .astype(NPBF16),
        ones1=np.ones((1, P), NPBF16),
        attcol=np.abs(attf).reshape(P, 1).astype(NPF32),
        sgn4=sgn.astype(NPBF16),
        i128f=np.eye(P, dtype=NPF32),
        b1c=np.asarray(inputs["b1"][i]).reshape(P, 1).astype(NPF32),
        b2c=np.asarray(inputs["b2"][i]).reshape(P, 1).astype(NPF32),
        bgc=(np.asarray(inputs["bias"][i]) + np.asarray(inputs["bl"][i]))
        .reshape(P, 1)
        .astype(NPF32),
    )
    return m


_NC_CACHE = {}


def kernel(**inputs):
    nodes = np.asarray(inputs["nodes"], dtype=np.float32)
    edge_index = np.asarray(inputs["edge_index"])
    edge_attr = np.asarray(inputs["edge_attr"], dtype=np.float32)

    n, d = nodes.shape
    assert (n, d) == (N, D)
    nloc = n // NCORES
    nwin = -(-nloc // P)

    data, kt = _preprocess(edge_index, edge_attr, NCORES, nloc, nwin)

    key = (nwin, kt, NCORES)
    if key not in _NC_CACHE:
        _NC_CACHE[key] = build_layer_nc(dict(nwin=nwin, kt=kt, ncores=NCORES))
    nc = _NC_CACHE[key]

    x_curr = np.ascontiguousarray(nodes.T)

    for layer in range(L):
        wmap = _layer_weight_maps(inputs, layer, np.asarray(inputs["att"]))
        xce = x_curr.astype(NP_EDGE)
        in_maps = []
        for c in range(NCORES):
            base = c * nloc
            xTloc = x_curr[:, base + data[c]["nl_flat"]].copy()
            xTloc[:, ~data[c]["used"]] = 0.0
            m = dict(wmap)
            m["xgT"] = np.ascontiguousarray(xce[:, data[c]["src_slot"]])
            m["xTloc"] = xTloc.astype(NPBF16)
            m["seld"] = data[c]["seld"]
            m["selTd"] = data[c]["selTd"]
            m["eaT"] = data[c]["eaT"]
            in_maps.append(m)
        res = run_bass_kernel_spmd(
            nc, in_maps, core_ids=list(range(NCORES)), trace=TRACE
        )
        if res.exec_time_ns is not None:
            LAST_EXEC_NS.append(res.exec_time_ns)
        if TRACE:
            LAST_RESULTS.append(res)
        outs = res.results
        x_next = np.zeros((P, n), NPF32)
        for c in range(NCORES):
            xo = outs[c]["xoutT"]
            u = data[c]["used"]
            x_next[:, c * nloc + data[c]["nl_flat"][u]] = xo[:, u]
        x_curr = x_next

    return np.ascontiguousarray(x_curr.T.astype(np.float32))


# revision 13
# speedup vs baseline: 2.4826x; 1.0098x over previous
"""Bass/Trainium2 kernel for nn_BlockGNN (2-layer GATv2 + MLP) on 8 NeuronCores.

Known-good 871us configuration (run2): 128-dst windows, feature-major z,
ACT Prelu |att| trick, per-tile alpha mini-matmuls, per-window software
pipeline, f32 MLP.
"""

import os
import sys
import time

import numpy as np

os.environ.setdefault("MYCRO_LOCAL_CACHE", "1")

for _p in ("/opt/trn_rl_repo",):
    if os.path.isdir(_p) and _p not in sys.path:
        sys.path.append(_p)

import concourse.bass as bass
import concourse.bacc as bacc
import concourse.mybir as mybir
import concourse.tile as tile
from concourse.bass import AP
from concourse.bass_utils import run_bass_kernel_spmd

F32 = mybir.dt.float32
BF16 = mybir.dt.bfloat16
FP8 = mybir.dt.float8e4

NPF32 = np.float32
NPBF16 = mybir.dt.np(BF16)
NPFP8 = mybir.dt.np(FP8)

N, E, D, H, CDIM, L = 50000, 800000, 128, 4, 32, 2
P = 128
NCORES = 8
NEG = 0.2

EDGE_DT = BF16
NP_EDGE = NPBF16
SEL_DT = FP8
NP_SEL = NPFP8
MLP_DT = F32
NP_MLP = NPF32
TRACE = bool(int(os.environ.get("KTRACE", "0")))

LAST_EXEC_NS = []
LAST_RESULTS = []


def _install_ntff_hook():
    try:
        import antenv.axon_hooks  # noqa: F401
        return
    except ImportError:
        pass
    import contextlib
    import ctypes
    import types

    try:
        import antenv
    except ImportError:
        return
    so_path = "/opt/axon/libaxon_pjrt.so"
    if not os.path.exists(so_path):
        return
    lib = ctypes.CDLL(so_path)
    if not hasattr(lib, "axon_start_nrt_profile"):
        return
    lib.axon_start_nrt_profile.argtypes = [
        ctypes.POINTER(ctypes.c_int64),
        ctypes.c_size_t,
    ]
    lib.axon_start_nrt_profile.restype = ctypes.c_int64
    lib.axon_stop_nrt_profile.argtypes = [ctypes.c_char_p]
    lib.axon_stop_nrt_profile.restype = ctypes.c_int64

    @contextlib.contextmanager
    def _hook(output_dir, device_ids):
        import jax

        jax.devices()
        if device_ids:
            ids = (ctypes.c_int64 * len(device_ids))(*device_ids)
            rc = lib.axon_start_nrt_profile(ids, len(device_ids))
        else:
            rc = lib.axon_start_nrt_profile(None, 0)
        if rc != 0:
            raise RuntimeError(f"axon_start_nrt_profile rc={rc}")
        try:
            yield
        finally:
            n = lib.axon_stop_nrt_profile(str(output_dir).encode())
            print(f"ntff profile: {n} file(s) -> {output_dir}", file=sys.stderr)

    mod = types.ModuleType("antenv.axon_hooks")
    _state = {"hook": _hook}
    mod.get_axon_ntff_profile_hook = lambda: _state["hook"]
    mod.set_axon_ntff_profile_hook = lambda h: _state.update(hook=h)
    sys.modules["antenv.axon_hooks"] = mod
    antenv.axon_hooks = mod


if TRACE:
    _install_ntff_hook()


def _bcast_last(ap: AP, n: int) -> AP:
    return AP(ap.tensor, ap.offset, [list(p) for p in ap.ap] + [[0, n]])


def build_layer_nc(cfg, enable_asserts=False):
    NWIN, KT = cfg["nwin"], cfg["kt"]
    NLOCP = NWIN * P
    ESLOT = NWIN * KT * P
    assert KT % 2 == 0
    MACROS = []
    j0 = 0
    while j0 < KT:
        wdt = 4 if KT - j0 >= 4 else KT - j0
        MACROS.append((j0, wdt))
        j0 += wdt
    NMAC = len(MACROS)
    assert NMAC >= 3

    nc = bacc.Bacc(
        "TRN2",
        target_bir_lowering=False,
        debug=False,
        enable_asserts=enable_asserts,
        num_devices=cfg.get("ncores", NCORES),
    )

    xgT = nc.dram_tensor("xgT", [P, ESLOT], EDGE_DT, kind="ExternalInput").ap()
    xTloc = nc.dram_tensor("xTloc", [P, NLOCP], BF16, kind="ExternalInput").ap()
    Wl_b = nc.dram_tensor("Wl_b", [P, P], EDGE_DT, kind="ExternalInput").ap()
    Wr = nc.dram_tensor("Wr", [P, P], BF16, kind="ExternalInput").ap()
    w1 = nc.dram_tensor("w1", [P, P], MLP_DT, kind="ExternalInput").ap()
    w2 = nc.dram_tensor("w2", [P, P], MLP_DT, kind="ExternalInput").ap()
    We_b = nc.dram_tensor("We_b", [CDIM, P], EDGE_DT, kind="ExternalInput").ap()
    attcol = nc.dram_tensor("attcol", [P, 1], F32, kind="ExternalInput").ap()
    pbias = nc.dram_tensor("pbias", [P, 1], F32, kind="ExternalInput").ap()
    sgn4 = nc.dram_tensor("sgn4", [P, 4], BF16, kind="ExternalInput").ap()
    i128f = nc.dram_tensor("i128f", [P, P], F32, kind="ExternalInput").ap()
    b1c = nc.dram_tensor("b1c", [P, 1], F32, kind="ExternalInput").ap()
    b2c = nc.dram_tensor("b2c", [P, 1], F32, kind="ExternalInput").ap()
    bgc = nc.dram_tensor("bgc", [P, 1], F32, kind="ExternalInput").ap()
    eaT = nc.dram_tensor("eaT", [CDIM, ESLOT], EDGE_DT, kind="ExternalInput").ap()
    seld = nc.dram_tensor("seld", [NWIN, P, KT * P], SEL_DT, kind="ExternalInput").ap()
    selTd = nc.dram_tensor("selTd", [NWIN, P, KT * P], SEL_DT, kind="ExternalInput").ap()
    xoutT = nc.dram_tensor("xoutT", [P, NLOCP], F32, kind="ExternalOutput").ap()

    AF = mybir.ActivationFunctionType
    OP = mybir.AluOpType

    with tile.TileContext(nc) as tc:
        with (
            tc.tile_pool(name="const", bufs=1) as cpool,
            tc.tile_pool(name="win", bufs=2) as wpool,
            tc.tile_pool(name="edge", bufs=3) as epool,
            tc.tile_pool(name="psZ", bufs=3, space="PSUM") as psZ,
            tc.tile_pool(name="psV", bufs=2, space="PSUM") as psV,
            tc.tile_pool(name="psM", bufs=1, space="PSUM") as psM,
            tc.tile_pool(name="psO", bufs=1, space="PSUM") as psO,
            tc.tile_pool(name="psE", bufs=1, space="PSUM") as psE,
        ):
            def cload(ap, shape, dt, tag):
                t = cpool.tile(shape, dt, tag=tag)
                nc.sync.dma_start(out=t[:], in_=ap)
                return t

            Wlb_s = cload(Wl_b, [P, P], EDGE_DT, tag="Wlb_s")
            Wr_s = cload(Wr, [P, P], BF16, tag="Wr_s")
            w1_s = cload(w1, [P, P], MLP_DT, tag="w1_s")
            w2_s = cload(w2, [P, P], MLP_DT, tag="w2_s")
            We_s = cload(We_b, [CDIM, P], EDGE_DT, tag="We_s")
            attc_s = cload(attcol, [P, 1], F32, tag="attc_s")
            pbias_s = cload(pbias, [P, 1], F32, tag="pbias_s")
            sgn_s = cload(sgn4, [P, 4], BF16, tag="sgn_s")
            i128f_s = cload(i128f, [P, P], F32, tag="i128f_s")
            b1c_s = cload(b1c, [P, 1], F32, tag="b1c_s")
            b2c_s = cload(b2c, [P, 1], F32, tag="b2c_s")
            bgc_s = cload(bgc, [P, 1], F32, tag="bgc_s")

            pending_epilogue = [None]

            def emit_epilogue():
                fn = pending_epilogue[0]
                if fn is not None:
                    pending_epilogue[0] = None
                    fn()

            for w in range(NWIN):
                xgT_sb = wpool.tile([P, KT * P], EDGE_DT, tag="xgT")
                nc.sync.dma_start(
                    out=xgT_sb[:], in_=xgT[:, w * KT * P : (w + 1) * KT * P]
                )
                eaT_sb = wpool.tile([CDIM, KT * P], EDGE_DT, tag="ea")
                nc.sync.dma_start(
                    out=eaT_sb[:], in_=eaT[:, w * KT * P : (w + 1) * KT * P]
                )
                sel_sb = wpool.tile([P, KT * P], SEL_DT, tag="sel")
                nc.sync.dma_start(out=sel_sb[:], in_=seld[w])
                selT_sb = wpool.tile([P, KT * P], SEL_DT, tag="selT")
                nc.sync.dma_start(out=selT_sb[:], in_=selTd[w])

                xtl_sb = wpool.tile([P, P], BF16, tag="xtl")
                nc.sync.dma_start(out=xtl_sb[:], in_=xTloc[:, w * P : (w + 1) * P])
                xr_ps = psE.tile([P, P], F32, tag="epi")
                nc.tensor.matmul(out=xr_ps[:], lhsT=xtl_sb[:], rhs=Wr_s[:],
                                 start=True, stop=True)
                xr_sb = wpool.tile([P, P], EDGE_DT, tag="xrs")
                nc.vector.tensor_copy(xr_sb[:], xr_ps[:])

                out12 = psO.tile([P, 132], F32, tag="o12")

                state = [None] * NMAC

                def stage_alpha(mi):
                    j0, MW, vq, am, comb = state[mi]
                    mini = psM.tile([P, 16], F32, tag="mini")
                    for u in range(MW):
                        nc.tensor.matmul(
                            out=mini[:, u * 4 : (u + 1) * 4],
                            lhsT=am[:, u * P : (u + 1) * P],
                            rhs=sgn_s[:],
                            start=(u == 0),
                            stop=(u == MW - 1),
                        )
                    comb_v = comb[:, : MW * 132].rearrange("p (b f) -> p b f",
                                                           f=132)
                    nc.scalar.activation(
                        comb_v[:, :, P : P + 4],
                        mini[:, : MW * 4].rearrange("p (b h) -> p b h", h=4),
                        AF.Exp,
                    )
                    nc.vector.tensor_tensor(
                        comb_v[:, :, 0:P].rearrange("p b (h c) -> p b h c", c=CDIM),
                        vq[:, : MW * P].rearrange("p (b h c) -> p b h c",
                                                  b=MW, c=CDIM),
                        _bcast_last(comb_v[:, :, P : P + 4], CDIM),
                        op=OP.mult,
                    )

                def stage_out12(mi):
                    j0, MW, vq, am, comb = state[mi]
                    for u in range(MW):
                        j = j0 + u
                        nc.tensor.matmul(
                            out=out12[:],
                            lhsT=sel_sb[:, j * P : (j + 1) * P],
                            rhs=comb[:, u * 132 : (u + 1) * 132],
                            start=(mi == 0 and u == 0),
                            stop=(mi == NMAC - 1 and u == MW - 1),
                        )

                for mi, (j0, MW) in enumerate(MACROS):
                    S = MW * P
                    zq = psZ.tile([P, 4 * P], F32, tag="zq")
                    nc.tensor.matmul(
                        out=zq[:, :S],
                        lhsT=We_s[:],
                        rhs=eaT_sb[:, j0 * P : j0 * P + S],
                        start=True, stop=False,
                    )
                    nc.tensor.matmul(
                        out=zq[:, :S],
                        lhsT=Wlb_s[:],
                        rhs=xgT_sb[:, j0 * P : j0 * P + S],
                        start=False, stop=False,
                    )
                    nc.tensor.matmul(
                        out=zq[:, :S],
                        lhsT=xr_sb[:],
                        rhs=selT_sb[:, j0 * P : j0 * P + S],
                        start=False, stop=True,
                    )
                    am = epool.tile([P, 4 * P], EDGE_DT, tag="am")
                    nc.scalar.activation(am[:, :S], zq[:, :S], AF.Prelu,
                                         scale=attc_s[:], bias=pbias_s[:],
                                         alpha=NEG)
                    vq = psV.tile([P, 4 * P], F32, tag="vq")
                    for u in range(MW):
                        j = j0 + u
                        nc.tensor.matmul(
                            out=vq[:, u * P : (u + 1) * P],
                            lhsT=xgT_sb[:, j * P : (j + 1) * P],
                            rhs=Wlb_s[:],
                            start=(u == 0),
                            stop=(u == MW - 1),
                        )
                    comb = epool.tile([P, 4 * 132], EDGE_DT, tag="comb")
                    state[mi] = (j0, MW, vq, am, comb)

                    if mi == 0:
                        emit_epilogue()
                    if mi >= 1:
                        stage_alpha(mi - 1)
                    if mi >= 2:
                        stage_out12(mi - 2)

                stage_alpha(NMAC - 1)
                stage_out12(NMAC - 2)
                stage_out12(NMAC - 1)

                def make_epilogue(w, out12):
                    def epi():
                        de = wpool.tile([P, 4], F32, tag="de")
                        nc.vector.tensor_scalar(de[:], out12[:, P : P + 4],
                                                1e-16, None, OP.add)
                        rc = wpool.tile([P, 4], F32, tag="rc")
                        nc.vector.reciprocal(rc[:], de[:])
                        gat = wpool.tile([P, P], F32, tag="gat")
                        for h in range(H):
                            nc.vector.tensor_scalar(
                                gat[:, h * CDIM : (h + 1) * CDIM],
                                out12[:, h * CDIM : (h + 1) * CDIM],
                                rc[:, h : h + 1],
                                None,
                                OP.mult,
                            )
                        gatT_ps = psE.tile([P, P], F32, tag="epi")
                        nc.tensor.transpose(gatT_ps[:], gat[:], i128f_s[:])
                        gTb = wpool.tile([P, P], MLP_DT, tag="gTb")
                        nc.vector.tensor_scalar(gTb[:], gatT_ps[:], bgc_s[:],
                                                None, OP.add)
                        y1_ps = psE.tile([P, P], F32, tag="epi")
                        nc.tensor.matmul(out=y1_ps[:], lhsT=w1_s[:], rhs=gTb[:],
                                         start=True, stop=True)
                        y1s = wpool.tile([P, P], MLP_DT, tag="y1s")
                        nc.scalar.activation(y1s[:], y1_ps[:], AF.Relu,
                                             bias=b1c_s[:])
                        y2_ps = psE.tile([P, P], F32, tag="epi")
                        nc.tensor.matmul(out=y2_ps[:], lhsT=w2_s[:], rhs=y1s[:],
                                         start=True, stop=True)
                        xo = wpool.tile([P, P], F32, tag="xo")
                        nc.vector.tensor_scalar(xo[:], y2_ps[:], b2c_s[:],
                                                None, OP.add)
                        nc.sync.dma_start(out=xoutT[:, w * P : (w + 1) * P],
                                          in_=xo[:])
                    return epi

                pending_epilogue[0] = make_epilogue(w, out12)

            emit_epilogue()

    nc.compile()
    return nc


def _preprocess(edge_index, edge_attr, ncores, nloc, nwin):
    src = np.ascontiguousarray(edge_index[0]).astype(np.int64)
    dst = np.ascontiguousarray(edge_index[1]).astype(np.int64)
    n = nloc * ncores
    ea = np.ascontiguousarray(edge_attr, dtype=np.float32)

    deg = np.bincount(dst, minlength=n).astype(np.float32)
    order = np.argsort(dst, kind="stable")
    dst_s = dst[order]
    src_s = src[order]
    ea_s = ea[order]
    cs = np.concatenate(
        [np.zeros((1, ea.shape[1]), np.float64), np.cumsum(ea_s, 0, dtype=np.float64)]
    )
    starts = np.searchsorted(dst_s, np.arange(n))
    ends = np.searchsorted(dst_s, np.arange(n) + 1)
    loop_attr = ((cs[ends] - cs[starts]) / np.maximum(deg, 1.0)[:, None]).astype(
        np.float32
    )

    import heapq

    cores = []
    maxcnt = 0
    for c in range(ncores):
        base = c * nloc
        lo, hi = starts[base], ends[base + nloc - 1]
        s2 = np.concatenate([src_s[lo:hi], np.arange(base, base + nloc)])
        dl = np.concatenate([dst_s[lo:hi], np.arange(base, base + nloc)]) - base
        e2 = np.concatenate([ea_s[lo:hi], loop_attr[base : base + nloc]], 0)

        w_of = np.empty(nloc, np.int64)
        pos_of = np.empty(nloc, np.int64)
        wdeg = (deg[base : base + nloc] + 1.0).astype(np.int64)
        heap = [(0, w, 0) for w in range(nwin)]
        heapq.heapify(heap)
        for node in np.argsort(-wdeg):
            tot, w, fill = heapq.heappop(heap)
            w_of[node] = w
            pos_of[node] = fill
            fill += 1
            tot += int(wdeg[node])
            if fill < P:
                heapq.heappush(heap, (tot, w, fill))
            else:
                heapq.heappush(heap, (1 << 60, w, fill))
        we = w_of[dl]
        pe_ = pos_of[dl]
        o = np.argsort(we, kind="stable")
        s2, e2, we, pe_ = s2[o], e2[o], we[o], pe_[o]
        wstart = np.searchsorted(we, np.arange(nwin))
        wend = np.searchsorted(we, np.arange(nwin) + 1)
        cnts = wend - wstart
        maxcnt = max(maxcnt, int(cnts.max()))
        nl_flat = np.zeros(nwin * P, np.int64)
        nl_flat[w_of * P + pos_of] = np.arange(nloc)
        used = np.zeros(nwin * P, bool)
        used[w_of * P + pos_of] = True
        cores.append((s2, e2, pe_, wstart, cnts, nl_flat, used))

    kt = -(-maxcnt // P)
    if kt % 2:
        kt += 1
    S = kt * P

    data = []
    for (s2, e2, pe_, wstart, cnts, nl_flat, used) in cores:
        nslot = nwin * S
        src_slot = np.zeros(nslot, np.int64)
        dstw_slot = np.full(nslot, -1, np.int64)
        ea_slot = np.zeros((nslot, CDIM), np.float32)
        idx = np.concatenate([np.arange(cnts[w]) + w * S for w in range(nwin)])
        src_slot[idx] = s2
        dstw_slot[idx] = pe_
        ea_slot[idx] = e2

        dw = dstw_slot.reshape(nwin, kt, P)
        sel = (dw[:, :, :, None] == np.arange(P)[None, None, None, :])
        sel = sel.transpose(0, 2, 1, 3).reshape(nwin, P, kt * P).astype(NP_SEL)
        selT = (dw[:, :, None, :] == np.arange(P)[None, None, :, None])
        selT = selT.transpose(0, 2, 1, 3).reshape(nwin, P, kt * P).astype(NP_SEL)
        eaT = np.ascontiguousarray(ea_slot.T).astype(NP_EDGE)
        data.append(dict(src_slot=src_slot, seld=sel, selTd=selT, eaT=eaT,
                         nl_flat=nl_flat, used=used))
    return data, kt


def _layer_weight_maps(inputs, layer, att):
    i = layer
    attf = att[i].reshape(-1).astype(np.float32)
    sgn = np.zeros((P, H), np.float32)
    for h in range(H):
        sgn[h * CDIM : (h + 1) * CDIM, h] = np.sign(
            attf[h * CDIM : (h + 1) * CDIM]
        )
    m = dict(
        Wl_b=np.ascontiguousarray(inputs["Wl"][i]).astype(NP_EDGE),
        Wr=np.ascontiguousarray(inputs["Wr"][i]).astype(NPBF16),
        w1=np.ascontiguousarray(inputs["w1"][i]).astype(NP_MLP),
        w2=np.ascontiguousarray(inputs["w2"][i]).astype(NP_MLP),
        We_b=np.ascontiguousarray(inputs["We"][i]).astype(NP_EDGE),
        attcol=np.abs(attf).reshape(P, 1).astype(NPF32),
        pbias=(np.abs(attf)
               * (np.asarray(inputs["br"][i]) + np.asarray(inputs["bl"][i])))
        .reshape(P, 1)
        .astype(NPF32),
        sgn4=sgn.astype(NPBF16),
        i128f=np.eye(P, dtype=NPF32),
        b1c=np.asarray(inputs["b1"][i]).reshape(P, 1).astype(NPF32),
        b2c=np.asarray(inputs["b2"][i]).reshape(P, 1).astype(NPF32),
        bgc=(np.asarray(inputs["bias"][i]) + np.asarray(inputs["bl"][i]))
        .reshape(P, 1)
        .astype(NPF32),
    )
    return m


_NC_CACHE = {}


def kernel(**inputs):
    nodes = np.asarray(inputs["nodes"], dtype=np.float32)
    edge_index = np.asarray(inputs["edge_index"])
    edge_attr = np.asarray(inputs["edge_attr"], dtype=np.float32)

    n, d = nodes.shape
    assert (n, d) == (N, D)
    nloc = n // NCORES
    nwin = -(-nloc // P)

    data, kt = _preprocess(edge_index, edge_attr, NCORES, nloc, nwin)

    key = (nwin, kt, NCORES)
    if key not in _NC_CACHE:
        _NC_CACHE[key] = build_layer_nc(dict(nwin=nwin, kt=kt, ncores=NCORES))
    nc = _NC_CACHE[key]

    x_curr = np.ascontiguousarray(nodes.T)

    for layer in range(L):
        wmap = _layer_weight_maps(inputs, layer, np.asarray(inputs["att"]))
        xce = x_curr.astype(NP_EDGE)
        in_maps = []
        for c in range(NCORES):
            base = c * nloc
            xTloc = x_curr[:, base + data[c]["nl_flat"]].copy()
            xTloc[:, ~data[c]["used"]] = 0.0
            m = dict(wmap)
            m["xgT"] = np.ascontiguousarray(xce[:, data[c]["src_slot"]])
            m["xTloc"] = xTloc.astype(NPBF16)
            m["seld"] = data[c]["seld"]
            m["selTd"] = data[c]["selTd"]
            m["eaT"] = data[c]["eaT"]
            in_maps.append(m)
        res = run_bass_kernel_spmd(
            nc, in_maps, core_ids=list(range(NCORES)), trace=TRACE
        )
        if res.exec_time_ns is not None:
            LAST_EXEC_NS.append(res.exec_time_ns)
        if TRACE:
            LAST_RESULTS.append(res)
        outs = res.results
        x_next = np.zeros((P, n), NPF32)
        for c in range(NCORES):
            xo = outs[c]["xoutT"]
            u = data[c]["used"]
            x_next[:, c * nloc + data[c]["nl_flat"][u]] = xo[:, u]
        x_curr = x_next

    return np.ascontiguousarray(x_curr.T.astype(np.float32))


# revision 14
# speedup vs baseline: 2.4878x; 1.0021x over previous
"""Bass/Trainium2 kernel for nn_BlockGNN (2-layer GATv2 + MLP) on 8 NeuronCores.

Known-good 871us configuration (run2): 128-dst windows, feature-major z,
ACT Prelu |att| trick, per-tile alpha mini-matmuls, per-window software
pipeline, f32 MLP.
"""

import os
import sys
import time

import numpy as np

os.environ.setdefault("MYCRO_LOCAL_CACHE", "1")

for _p in ("/opt/trn_rl_repo",):
    if os.path.isdir(_p) and _p not in sys.path:
        sys.path.append(_p)

import concourse.bass as bass
import concourse.bacc as bacc
import concourse.mybir as mybir
import concourse.tile as tile
from concourse.bass import AP
from concourse.bass_utils import run_bass_kernel_spmd

F32 = mybir.dt.float32
BF16 = mybir.dt.bfloat16
FP8 = mybir.dt.float8e4

NPF32 = np.float32
NPBF16 = mybir.dt.np(BF16)
NPFP8 = mybir.dt.np(FP8)

N, E, D, H, CDIM, L = 50000, 800000, 128, 4, 32, 2
P = 128
NCORES = 8
NEG = 0.2

EDGE_DT = BF16
NP_EDGE = NPBF16
SEL_DT = FP8
NP_SEL = NPFP8
MLP_DT = F32
NP_MLP = NPF32
TRACE = bool(int(os.environ.get("KTRACE", "0")))

LAST_EXEC_NS = []
LAST_RESULTS = []


def _install_ntff_hook():
    try:
        import antenv.axon_hooks  # noqa: F401
        return
    except ImportError:
        pass
    import contextlib
    import ctypes
    import types

    try:
        import antenv
    except ImportError:
        return
    so_path = "/opt/axon/libaxon_pjrt.so"
    if not os.path.exists(so_path):
        return
    lib = ctypes.CDLL(so_path)
    if not hasattr(lib, "axon_start_nrt_profile"):
        return
    lib.axon_start_nrt_profile.argtypes = [
        ctypes.POINTER(ctypes.c_int64),
        ctypes.c_size_t,
    ]
    lib.axon_start_nrt_profile.restype = ctypes.c_int64
    lib.axon_stop_nrt_profile.argtypes = [ctypes.c_char_p]
    lib.axon_stop_nrt_profile.restype = ctypes.c_int64

    @contextlib.contextmanager
    def _hook(output_dir, device_ids):
        import jax

        jax.devices()
        if device_ids:
            ids = (ctypes.c_int64 * len(device_ids))(*device_ids)
            rc = lib.axon_start_nrt_profile(ids, len(device_ids))
        else:
            rc = lib.axon_start_nrt_profile(None, 0)
        if rc != 0:
            raise RuntimeError(f"axon_start_nrt_profile rc={rc}")
        try:
            yield
        finally:
            n = lib.axon_stop_nrt_profile(str(output_dir).encode())
            print(f"ntff profile: {n} file(s) -> {output_dir}", file=sys.stderr)

    mod = types.ModuleType("antenv.axon_hooks")
    _state = {"hook": _hook}
    mod.get_axon_ntff_profile_hook = lambda: _state["hook"]
    mod.set_axon_ntff_profile_hook = lambda h: _state.update(hook=h)
    sys.modules["antenv.axon_hooks"] = mod
    antenv.axon_hooks = mod


if TRACE:
    _install_ntff_hook()


def _bcast_last(ap: AP, n: int) -> AP:
    return AP(ap.tensor, ap.offset, [list(p) for p in ap.ap] + [[0, n]])


def build_layer_nc(cfg, enable_asserts=False):
    NWIN, KT = cfg["nwin"], cfg["kt"]
    NLOCP = NWIN * P
    ESLOT = NWIN * KT * P
    assert KT % 2 == 0
    MACROS = []
    j0 = 0
    while j0 < KT:
        wdt = 4 if KT - j0 >= 4 else KT - j0
        MACROS.append((j0, wdt))
        j0 += wdt
    NMAC = len(MACROS)
    assert NMAC >= 3

    nc = bacc.Bacc(
        "TRN2",
        target_bir_lowering=False,
        debug=False,
        enable_asserts=enable_asserts,
        num_devices=cfg.get("ncores", NCORES),
    )

    xgT = nc.dram_tensor("xgT", [P, ESLOT], EDGE_DT, kind="ExternalInput").ap()
    xTloc = nc.dram_tensor("xTloc", [P, NLOCP], BF16, kind="ExternalInput").ap()
    Wl_b = nc.dram_tensor("Wl_b", [P, P], EDGE_DT, kind="ExternalInput").ap()
    Wr = nc.dram_tensor("Wr", [P, P], BF16, kind="ExternalInput").ap()
    w1 = nc.dram_tensor("w1", [P, P], MLP_DT, kind="ExternalInput").ap()
    w2 = nc.dram_tensor("w2", [P, P], MLP_DT, kind="ExternalInput").ap()
    We_b = nc.dram_tensor("We_b", [CDIM, P], EDGE_DT, kind="ExternalInput").ap()
    attcol = nc.dram_tensor("attcol", [P, 1], F32, kind="ExternalInput").ap()
    pbias = nc.dram_tensor("pbias", [P, 1], F32, kind="ExternalInput").ap()
    sgn4 = nc.dram_tensor("sgn4", [P, 4], BF16, kind="ExternalInput").ap()
    i128f = nc.dram_tensor("i128f", [P, P], F32, kind="ExternalInput").ap()
    b1c = nc.dram_tensor("b1c", [P, 1], F32, kind="ExternalInput").ap()
    b2c = nc.dram_tensor("b2c", [P, 1], F32, kind="ExternalInput").ap()
    bgc = nc.dram_tensor("bgc", [P, 1], F32, kind="ExternalInput").ap()
    eaT = nc.dram_tensor("eaT", [CDIM, ESLOT], EDGE_DT, kind="ExternalInput").ap()
    seld = nc.dram_tensor("seld", [NWIN, P, KT * P], SEL_DT, kind="ExternalInput").ap()
    selTd = nc.dram_tensor("selTd", [NWIN, P, KT * P], SEL_DT, kind="ExternalInput").ap()
    xoutT = nc.dram_tensor("xoutT", [P, NLOCP], F32, kind="ExternalOutput").ap()

    AF = mybir.ActivationFunctionType
    OP = mybir.AluOpType

    with tile.TileContext(nc) as tc:
        with (
            tc.tile_pool(name="const", bufs=1) as cpool,
            tc.tile_pool(name="win", bufs=2) as wpool,
            tc.tile_pool(name="edge", bufs=3) as epool,
            tc.tile_pool(name="psZ", bufs=2, space="PSUM") as psZ,
            tc.tile_pool(name="psV", bufs=3, space="PSUM") as psV,
            tc.tile_pool(name="psM", bufs=1, space="PSUM") as psM,
            tc.tile_pool(name="psO", bufs=1, space="PSUM") as psO,
            tc.tile_pool(name="psE", bufs=1, space="PSUM") as psE,
        ):
            def cload(ap, shape, dt, tag):
                t = cpool.tile(shape, dt, tag=tag)
                nc.sync.dma_start(out=t[:], in_=ap)
                return t

            Wlb_s = cload(Wl_b, [P, P], EDGE_DT, tag="Wlb_s")
            Wr_s = cload(Wr, [P, P], BF16, tag="Wr_s")
            w1_s = cload(w1, [P, P], MLP_DT, tag="w1_s")
            w2_s = cload(w2, [P, P], MLP_DT, tag="w2_s")
            We_s = cload(We_b, [CDIM, P], EDGE_DT, tag="We_s")
            attc_s = cload(attcol, [P, 1], F32, tag="attc_s")
            pbias_s = cload(pbias, [P, 1], F32, tag="pbias_s")
            sgn_s = cload(sgn4, [P, 4], BF16, tag="sgn_s")
            i128f_s = cload(i128f, [P, P], F32, tag="i128f_s")
            b1c_s = cload(b1c, [P, 1], F32, tag="b1c_s")
            b2c_s = cload(b2c, [P, 1], F32, tag="b2c_s")
            bgc_s = cload(bgc, [P, 1], F32, tag="bgc_s")

            pending_epilogue = [None]

            def emit_epilogue():
                fn = pending_epilogue[0]
                if fn is not None:
                    pending_epilogue[0] = None
                    fn()

            for w in range(NWIN):
                xgT_sb = wpool.tile([P, KT * P], EDGE_DT, tag="xgT")
                nc.sync.dma_start(
                    out=xgT_sb[:], in_=xgT[:, w * KT * P : (w + 1) * KT * P]
                )
                eaT_sb = wpool.tile([CDIM, KT * P], EDGE_DT, tag="ea")
                nc.sync.dma_start(
                    out=eaT_sb[:], in_=eaT[:, w * KT * P : (w + 1) * KT * P]
                )
                sel_sb = wpool.tile([P, KT * P], SEL_DT, tag="sel")
                nc.sync.dma_start(out=sel_sb[:], in_=seld[w])
                selT_sb = wpool.tile([P, KT * P], SEL_DT, tag="selT")
                nc.sync.dma_start(out=selT_sb[:], in_=selTd[w])

                xtl_sb = wpool.tile([P, P], BF16, tag="xtl")
                nc.sync.dma_start(out=xtl_sb[:], in_=xTloc[:, w * P : (w + 1) * P])
                xr_ps = psE.tile([P, P], F32, tag="epi")
                nc.tensor.matmul(out=xr_ps[:], lhsT=xtl_sb[:], rhs=Wr_s[:],
                                 start=True, stop=True)
                xr_sb = wpool.tile([P, P], EDGE_DT, tag="xrs")
                nc.vector.tensor_copy(xr_sb[:], xr_ps[:])

                out12 = psO.tile([P, 132], F32, tag="o12")

                state = [None] * NMAC

                def stage_alpha(mi):
                    j0, MW, vq, am, comb = state[mi]
                    mini = psM.tile([P, 16], F32, tag="mini")
                    for u in range(MW):
                        nc.tensor.matmul(
                            out=mini[:, u * 4 : (u + 1) * 4],
                            lhsT=am[:, u * P : (u + 1) * P],
                            rhs=sgn_s[:],
                            start=(u == 0),
                            stop=(u == MW - 1),
                        )
                    comb_v = comb[:, : MW * 132].rearrange("p (b f) -> p b f",
                                                           f=132)
                    nc.scalar.activation(
                        comb_v[:, :, P : P + 4],
                        mini[:, : MW * 4].rearrange("p (b h) -> p b h", h=4),
                        AF.Exp,
                    )
                    nc.vector.tensor_tensor(
                        comb_v[:, :, 0:P].rearrange("p b (h c) -> p b h c", c=CDIM),
                        vq[:, : MW * P].rearrange("p (b h c) -> p b h c",
                                                  b=MW, c=CDIM),
                        _bcast_last(comb_v[:, :, P : P + 4], CDIM),
                        op=OP.mult,
                    )

                def stage_out12(mi):
                    j0, MW, vq, am, comb = state[mi]
                    for u in range(MW):
                        j = j0 + u
                        nc.tensor.matmul(
                            out=out12[:],
                            lhsT=sel_sb[:, j * P : (j + 1) * P],
                            rhs=comb[:, u * 132 : (u + 1) * 132],
                            start=(mi == 0 and u == 0),
                            stop=(mi == NMAC - 1 and u == MW - 1),
                        )

                for mi, (j0, MW) in enumerate(MACROS):
                    S = MW * P
                    zq = psZ.tile([P, 4 * P], F32, tag="zq")
                    nc.tensor.matmul(
                        out=zq[:, :S],
                        lhsT=Wlb_s[:],
                        rhs=xgT_sb[:, j0 * P : j0 * P + S],
                        start=True, stop=False,
                    )
                    nc.tensor.matmul(
                        out=zq[:, :S],
                        lhsT=We_s[:],
                        rhs=eaT_sb[:, j0 * P : j0 * P + S],
                        start=False, stop=False,
                    )
                    nc.tensor.matmul(
                        out=zq[:, :S],
                        lhsT=xr_sb[:],
                        rhs=selT_sb[:, j0 * P : j0 * P + S],
                        start=False, stop=True,
                    )
                    am = epool.tile([P, 4 * P], EDGE_DT, tag="am")
                    nc.scalar.activation(am[:, :S], zq[:, :S], AF.Prelu,
                                         scale=attc_s[:], bias=pbias_s[:],
                                         alpha=NEG)
                    vq = psV.tile([P, 4 * P], F32, tag="vq")
                    for u in range(MW):
                        j = j0 + u
                        nc.tensor.matmul(
                            out=vq[:, u * P : (u + 1) * P],
                            lhsT=xgT_sb[:, j * P : (j + 1) * P],
                            rhs=Wlb_s[:],
                            start=(u == 0),
                            stop=(u == MW - 1),
                        )
                    comb = epool.tile([P, 4 * 132], EDGE_DT, tag="comb")
                    state[mi] = (j0, MW, vq, am, comb)

                    if mi == 0:
                        emit_epilogue()
                    if mi >= 1:
                        stage_alpha(mi - 1)
                    if mi >= 2:
                        stage_out12(mi - 2)

                stage_alpha(NMAC - 1)
                stage_out12(NMAC - 2)
                stage_out12(NMAC - 1)

                def make_epilogue(w, out12):
                    def epi():
                        de = wpool.tile([P, 4], F32, tag="de")
                        nc.vector.tensor_scalar(de[:], out12[:, P : P + 4],
                                                1e-16, None, OP.add)
                        rc = wpool.tile([P, 4], F32, tag="rc")
                        nc.vector.reciprocal(rc[:], de[:])
                        gat = wpool.tile([P, P], F32, tag="gat")
                        for h in range(H):
                            nc.vector.tensor_scalar(
                                gat[:, h * CDIM : (h + 1) * CDIM],
                                out12[:, h * CDIM : (h + 1) * CDIM],
                                rc[:, h : h + 1],
                                None,
                                OP.mult,
                            )
                        gatT_ps = psE.tile([P, P], F32, tag="epi")
                        nc.tensor.transpose(gatT_ps[:], gat[:], i128f_s[:])
                        gTb = wpool.tile([P, P], MLP_DT, tag="gTb")
                        nc.vector.tensor_scalar(gTb[:], gatT_ps[:], bgc_s[:],
                                                None, OP.add)
                        y1_ps = psE.tile([P, P], F32, tag="epi")
                        nc.tensor.matmul(out=y1_ps[:], lhsT=w1_s[:], rhs=gTb[:],
                                         start=True, stop=True)
                        y1s = wpool.tile([P, P], MLP_DT, tag="y1s")
                        nc.scalar.activation(y1s[:], y1_ps[:], AF.Relu,
                                             bias=b1c_s[:])
                        y2_ps = psE.tile([P, P], F32, tag="epi")
                        nc.tensor.matmul(out=y2_ps[:], lhsT=w2_s[:], rhs=y1s[:],
                                         start=True, stop=True)
                        xo = wpool.tile([P, P], F32, tag="xo")
                        nc.vector.tensor_scalar(xo[:], y2_ps[:], b2c_s[:],
                                                None, OP.add)
                        nc.sync.dma_start(out=xoutT[:, w * P : (w + 1) * P],
                                          in_=xo[:])
                    return epi

                pending_epilogue[0] = make_epilogue(w, out12)

            emit_epilogue()

    nc.compile()
    return nc


def _preprocess(edge_index, edge_attr, ncores, nloc, nwin):
    src = np.ascontiguousarray(edge_index[0]).astype(np.int64)
    dst = np.ascontiguousarray(edge_index[1]).astype(np.int64)
    n = nloc * ncores
    ea = np.ascontiguousarray(edge_attr, dtype=np.float32)

    deg = np.bincount(dst, minlength=n).astype(np.float32)
    order = np.argsort(dst, kind="stable")
    dst_s = dst[order]
    src_s = src[order]
    ea_s = ea[order]
    cs = np.concatenate(
        [np.zeros((1, ea.shape[1]), np.float64), np.cumsum(ea_s, 0, dtype=np.float64)]
    )
    starts = np.searchsorted(dst_s, np.arange(n))
    ends = np.searchsorted(dst_s, np.arange(n) + 1)
    loop_attr = ((cs[ends] - cs[starts]) / np.maximum(deg, 1.0)[:, None]).astype(
        np.float32
    )

    import heapq

    cores = []
    maxcnt = 0
    for c in range(ncores):
        base = c * nloc
        lo, hi = starts[base], ends[base + nloc - 1]
        s2 = np.concatenate([src_s[lo:hi], np.arange(base, base + nloc)])
        dl = np.concatenate([dst_s[lo:hi], np.arange(base, base + nloc)]) - base
        e2 = np.concatenate([ea_s[lo:hi], loop_attr[base : base + nloc]], 0)

        w_of = np.empty(nloc, np.int64)
        pos_of = np.empty(nloc, np.int64)
        wdeg = (deg[base : base + nloc] + 1.0).astype(np.int64)
        heap = [(0, w, 0) for w in range(nwin)]
        heapq.heapify(heap)
        for node in np.argsort(-wdeg):
            tot, w, fill = heapq.heappop(heap)
            w_of[node] = w
            pos_of[node] = fill
            fill += 1
            tot += int(wdeg[node])
            if fill < P:
                heapq.heappush(heap, (tot, w, fill))
            else:
                heapq.heappush(heap, (1 << 60, w, fill))
        we = w_of[dl]
        pe_ = pos_of[dl]
        o = np.argsort(we, kind="stable")
        s2, e2, we, pe_ = s2[o], e2[o], we[o], pe_[o]
        wstart = np.searchsorted(we, np.arange(nwin))
        wend = np.searchsorted(we, np.arange(nwin) + 1)
        cnts = wend - wstart
        maxcnt = max(maxcnt, int(cnts.max()))
        nl_flat = np.zeros(nwin * P, np.int64)
        nl_flat[w_of * P + pos_of] = np.arange(nloc)
        used = np.zeros(nwin * P, bool)
        used[w_of * P + pos_of] = True
        cores.append((s2, e2, pe_, wstart, cnts, nl_flat, used))

    kt = -(-maxcnt // P)
    if kt % 2:
        kt += 1
    S = kt * P

    data = []
    for (s2, e2, pe_, wstart, cnts, nl_flat, used) in cores:
        nslot = nwin * S
        src_slot = np.zeros(nslot, np.int64)
        dstw_slot = np.full(nslot, -1, np.int64)
        ea_slot = np.zeros((nslot, CDIM), np.float32)
        idx = np.concatenate([np.arange(cnts[w]) + w * S for w in range(nwin)])
        src_slot[idx] = s2
        dstw_slot[idx] = pe_
        ea_slot[idx] = e2

        dw = dstw_slot.reshape(nwin, kt, P)
        sel = (dw[:, :, :, None] == np.arange(P)[None, None, None, :])
        sel = sel.transpose(0, 2, 1, 3).reshape(nwin, P, kt * P).astype(NP_SEL)
        selT = (dw[:, :, None, :] == np.arange(P)[None, None, :, None])
        selT = selT.transpose(0, 2, 1, 3).reshape(nwin, P, kt * P).astype(NP_SEL)
        eaT = np.ascontiguousarray(ea_slot.T).astype(NP_EDGE)
        data.append(dict(src_slot=src_slot, seld=sel, selTd=selT, eaT=eaT,
                         nl_flat=nl_flat, used=used))
    return data, kt


def _layer_weight_maps(inputs, layer, att):
    i = layer
    attf = att[i].reshape(-1).astype(np.float32)
    sgn = np.zeros((P, H), np.float32)
    for h in range(H):
        sgn[h * CDIM : (h + 1) * CDIM, h] = np.sign(
            attf[h * CDIM : (h + 1) * CDIM]
        )
    m = dict(
        Wl_b=np.ascontiguousarray(inputs["Wl"][i]).astype(NP_EDGE),
        Wr=np.ascontiguousarray(inputs["Wr"][i]).astype(NPBF16),
        w1=np.ascontiguousarray(inputs["w1"][i]).astype(NP_MLP),
        w2=np.ascontiguousarray(inputs["w2"][i]).astype(NP_MLP),
        We_b=np.ascontiguousarray(inputs["We"][i]).astype(NP_EDGE),
        attcol=np.abs(attf).reshape(P, 1).astype(NPF32),
        pbias=(np.abs(attf)
               * (np.asarray(inputs["br"][i]) + np.asarray(inputs["bl"][i])))
        .reshape(P, 1)
        .astype(NPF32),
        sgn4=sgn.astype(NPBF16),
        i128f=np.eye(P, dtype=NPF32),
        b1c=np.asarray(inputs["b1"][i]).reshape(P, 1).astype(NPF32),
        b2c=np.asarray(inputs["b2"][i]).reshape(P, 1).astype(NPF32),
        bgc=(np.asarray(inputs["bias"][i]) + np.asarray(inputs["bl"][i]))
        .reshape(P, 1)
        .astype(NPF32),
    )
    return m


_NC_CACHE = {}


def kernel(**inputs):
    nodes = np.asarray(inputs["nodes"], dtype=np.float32)
    edge_index = np.asarray(inputs["edge_index"])
    edge_attr = np.asarray(inputs["edge_attr"], dtype=np.float32)

    n, d = nodes.shape
    assert (n, d) == (N, D)
    nloc = n // NCORES
    nwin = -(-nloc // P)

    data, kt = _preprocess(edge_index, edge_attr, NCORES, nloc, nwin)

    key = (nwin, kt, NCORES)
    if key not in _NC_CACHE:
        _NC_CACHE[key] = build_layer_nc(dict(nwin=nwin, kt=kt, ncores=NCORES))
    nc = _NC_CACHE[key]

    x_curr = np.ascontiguousarray(nodes.T)

    for layer in range(L):
        wmap = _layer_weight_maps(inputs, layer, np.asarray(inputs["att"]))
        xce = x_curr.astype(NP_EDGE)
        in_maps = []
        for c in range(NCORES):
            base = c * nloc
            xTloc = x_curr[:, base + data[c]["nl_flat"]].copy()
            xTloc[:, ~data[c]["used"]] = 0.0
            m = dict(wmap)
            m["xgT"] = np.ascontiguousarray(xce[:, data[c]["src_slot"]])
            m["xTloc"] = xTloc.astype(NPBF16)
            m["seld"] = data[c]["seld"]
            m["selTd"] = data[c]["selTd"]
            m["eaT"] = data[c]["eaT"]
            in_maps.append(m)
        res = run_bass_kernel_spmd(
            nc, in_maps, core_ids=list(range(NCORES)), trace=TRACE
        )
        if res.exec_time_ns is not None:
            LAST_EXEC_NS.append(res.exec_time_ns)
        if TRACE:
            LAST_RESULTS.append(res)
        outs = res.results
        x_next = np.zeros((P, n), NPF32)
        for c in range(NCORES):
            xo = outs[c]["xoutT"]
            u = data[c]["used"]
            x_next[:, c * nloc + data[c]["nl_flat"][u]] = xo[:, u]
        x_curr = x_next

    return np.ascontiguousarray(x_curr.T.astype(np.float32))
